# revision 41
# baseline (speedup 1.0000x reference)
"""DeltaNet forward Trainium2 kernel (8-core SPMD, batch x head-pair sharded).

Sharding: core c -> batch b=c//2, head-group hg=c%2 (heads 2hg, 2hg+1 = columns
hg*512 : hg*512+512 of the q/k/v/o projections).  Each core computes a partial
[L, D] output (its two heads' contribution through the output projection); the
host sums the two partials per batch.  norm_w is folded into Wo on the host.

Per-core pipeline, software-pipelined at emission level: the projection work
for strip s+1 is queued as ~20 filler closures and drained at designated
stall points inside strip s's scan chunks, so the (in-order) PE stream has
gap-filler matmuls wherever the scan's cross-engine handoffs would stall it.

  x^T strip (DMA xbar-transposed load, bf16)
  -> q/k/v projections channel-major z^T = W^T x^T (bf16 matmuls)
  -> depthwise causal conv = 4 accumulating diag(w_tap) matmuls (bf16)
  -> silu as z * sigmoid(z): sigmoid on Act engine, multiply on Pool engine
     (keeps the Act engine on a single act-function table all kernel long)
  -> l2norm scales via ones-matmul column sums; rsqrt via DVE quake+Newton
  -> per 128-token chunk: delta-rule scan in bf16; (I+A)^-1 by Neumann product
     form; S in fp32 master + bf16 working copy
  -> RMS-norm (DVE rsqrt) + swish gate, output projection (fp32r)
"""

import sys

sys.path.insert(0, "/opt/trn_rl_repo")

from collections import deque
from contextlib import ExitStack

import numpy as np

import concourse.bass as bass
import concourse.tile as tile
from concourse import bacc, mybir
from concourse.bass_utils import run_bass_kernel_spmd

FP = mybir.dt.float32
FR = mybir.dt.float32r
BF = mybir.dt.bfloat16
I32 = mybir.dt.int32
AF = mybir.ActivationFunctionType
OP = mybir.AluOpType

B, L, D, H = 4, 4096, 1024, 4
Dh = 256          # head dim
DL = 512          # per-core channels (2 heads)
KT = 4            # conv taps
C = 128           # chunk length (our choice; the math is chunk-size invariant)
LT = 512          # L-strip size
NS = L // LT      # 8 strips
CPS = LT // C     # 4 chunks per strip
NLEV = 2          # Neumann levels: exact would be 6 (A^64); A is
                  # strongly contractive here (||A||~0.5), A^8 ~ 0
EPS_RMS = 1e-5
EPS_L2 = 1e-12
QK3 = 0x5F3759DF  # quake fast-rsqrt seed constant


def deltanet_core(ctx: ExitStack, tc: tile.TileContext, io: dict):
    nc = tc.nc
    x, wq, wk, wv, wg, bt, wo, cq, ck, cv, out = (
        io["x"], io["wq"], io["wk"], io["wv"], io["wg"], io["bt"], io["wo"],
        io["cq"], io["ck"], io["cv"], io["out"])
    bcd = io["bcd"]

    pool1 = ctx.enter_context(tc.tile_pool(name="consts", bufs=1))
    xpool = ctx.enter_context(tc.tile_pool(name="xp", bufs=2))
    zpool = ctx.enter_context(tc.tile_pool(name="zp", bufs=4))
    stripT = ctx.enter_context(tc.tile_pool(name="stripT", bufs=1))
    stripD = ctx.enter_context(tc.tile_pool(name="stripD", bufs=2))
    small = ctx.enter_context(tc.tile_pool(name="small", bufs=2))
    hot = ctx.enter_context(tc.tile_pool(name="hot", bufs=5))
    spool = ctx.enter_context(tc.tile_pool(name="state", bufs=1))
    psA = ctx.enter_context(tc.tile_pool(name="psA", bufs=2, space="PSUM"))
    psS = ctx.enter_context(tc.tile_pool(name="psS", bufs=6, space="PSUM"))

    # ---- resident constants -------------------------------------------------
    # wq first, then strip 0's x, so the first projection can start ~6us in
    # instead of waiting for the whole constant working set to land.
    wt = {}
    x0_tiles = {}
    for name, w in (("q", wq),):
        wr = w.rearrange("(t p) n -> p t n", p=128)
        for kt8 in range(8):
            t = pool1.tile([128, 512], BF, tag=f"w{name}{kt8}",
                           name=f"w{name}{kt8}")
            nc.sync.dma_start(t[:], wr[:, kt8, :])
            wt[(name, kt8)] = t
    xr0 = x.rearrange("(t p) l -> p t l", p=128)
    for kt8 in range(8):
        xk = xpool.tile([128, LT], BF, tag=f"xt{kt8}", name=f"xt{kt8}")
        # Act DMA ring: runs concurrently with the weight loads on SP's ring
        nc.scalar.dma_start(out=xk[:], in_=xr0[:, kt8, 0:LT])
        x0_tiles[("x", kt8)] = xk
    cw = {}
    for name, cz in (("q", cq), ("k", ck), ("v", cv)):
        t = pool1.tile([128, 4, KT], FP, tag=f"c{name}")
        nc.sync.dma_start(t[:], cz.rearrange("(t p) j -> p t j", p=128))
        cw[name] = t
    bt0 = pool1.tile([128, CPS, 2], FP, tag="bt0")
    nc.sync.dma_start(bt0[:], bt[:, 0])
    for name, w in (("k", wk), ("v", wv), ("g", wg)):
        wr = w.rearrange("(t p) n -> p t n", p=128)
        for kt8 in range(8):
            t = pool1.tile([128, 512], BF, tag=f"w{name}{kt8}",
                           name=f"w{name}{kt8}")
            nc.sync.dma_start(t[:], wr[:, kt8, :])
            wt[(name, kt8)] = t
    wo_t = pool1.tile([128, 4, 1024], BF, tag="wo")
    nc.sync.dma_start(wo_t[:], wo.rearrange("(t p) n -> p t n", p=128))

    # mask_ua[:, 0, :] strict-upper, mask_ua[:, 1, :] inclusive-upper:
    # keep where x - a - y < 0  (a=0 -> x<y, a=1 -> x<=y)
    mask_ua = pool1.tile([128, 2, 128], FP, tag="mask_ua")
    nc.gpsimd.memset(mask_ua, 1.0)
    nc.gpsimd.affine_select(out=mask_ua[:], in_=mask_ua[:],
                            compare_op=OP.is_gt, fill=0.0, base=0,
                            pattern=[[1, 2], [1, 128]], channel_multiplier=-1)
    mask_sl = pool1.tile([128, 128], FP, tag="mask_sl")
    nc.gpsimd.memset(mask_sl, 1.0)
    nc.gpsimd.affine_select(out=mask_sl[:], in_=mask_sl[:],
                            compare_op=OP.is_gt, fill=0.0, base=0,
                            pattern=[[-1, 128]], channel_multiplier=1)
    from concourse.masks import make_identity
    ident_b = pool1.tile([128, 128], BF, tag="identb")
    make_identity(nc, ident_b)

    ones_col = pool1.tile([128, 1], BF, tag="ones_col")
    nc.vector.memset(ones_col, 1.0)
    ones2 = pool1.tile([128, 2], BF, tag="ones2")
    nc.vector.memset(ones2, 1.0)

    # conv diag tiles (bf16): diag(w_tap) per (tensor, ch-tile, tap).
    diag = {}
    for name in ("q", "k", "v"):
        dt = pool1.tile([128, 4, KT, 128], BF, tag=f"diag{name}")
        diag[name] = dt
        for ct in range(4):
            for j in range(KT):
                nc.vector.tensor_scalar_mul(dt[:, ct, j, :], ident_b[:],
                                            cw[name][:, ct, j:j + 1])

    # ---- persistent state ---------------------------------------------------
    # S32 ping-pongs between two buffers per head: the update writes the
    # other buffer (reading the old one), so chunk g+1's update never WARs
    # against chunk g's Sbf drain and the S critical chain is add-to-add.
    S32 = [[spool.tile([128, 2, 256], FP, name=f"S32_{h}{b}",
                       tag=f"S32_{h}{b}") for b in range(2)]
           for h in range(2)]
    Sbf = [spool.tile([128, 2, 256], BF, name=f"Sbf_{h}", tag=f"Sbf_{h}")
           for h in range(2)]
    for h in range(2):
        nc.vector.memset(S32[h][0], 0.0)
        nc.vector.memset(Sbf[h], 0.0)
    carries = {}
    for name in ("q", "k", "v"):
        for ct in range(4):
            cr = spool.tile([128, KT - 1], BF, tag=f"carry_{name}{ct}")
            nc.vector.memset(cr, 0.0)
            carries[(name, ct)] = cr

    # ---- DVE fast rsqrt (quake seed + Newton) ------------------------------
    # out = 1/sqrt(in*scale + bias); `inp` may live in PSUM.  Keeps the Act
    # engine free of Abs_reciprocal_sqrt so one act table serves the kernel.
    def dve_rsqrt(n, out_ap, inp, scale, bias, tag, iters, eng=None):
        # eng=nc.gpsimd runs the whole chain on Pool (inp must be SBUF)
        eng = eng or nc.vector
        xb = small.tile([128, n], FP, tag=f"{tag}_x", name=f"{tag}_x")
        eng.tensor_scalar(xb[:], inp, scale, bias,
                          op0=OP.mult, op1=OP.add)
        yb = small.tile([128, n], FP, tag=f"{tag}_y", name=f"{tag}_y")
        eng.tensor_scalar(yb.bitcast(I32)[:], xb.bitcast(I32)[:], 1,
                          None, op0=OP.logical_shift_right)
        eng.tensor_scalar(yb.bitcast(I32)[:], yb.bitcast(I32)[:], -1,
                          QK3, op0=OP.mult, op1=OP.add)
        for it in range(iters):
            hb = small.tile([128, n], FP, tag=f"{tag}_h{it}",
                            name=f"{tag}_h{it}")
            eng.tensor_tensor(hb[:], yb[:], yb[:], op=OP.mult)
            eng.tensor_tensor(hb[:], hb[:], xb[:], op=OP.mult)
            eng.tensor_scalar(hb[:], hb[:], -0.5, 1.5,
                              op0=OP.mult, op1=OP.add)
            if it == iters - 1:
                eng.tensor_tensor(out_ap, yb[:], hb[:], op=OP.mult)
            else:
                y2 = small.tile([128, n], FP, tag=f"{tag}_y{it}",
                                name=f"{tag}_y{it}")
                eng.tensor_tensor(y2[:], yb[:], hb[:], op=OP.mult)
                yb = y2

    # ---- filler queue: strip s+1 proj work drained inside strip s scan -----
    # Metered per chunk so chunk 3 (and the strip boundary) still has filler
    # matmuls instead of the queue draining greedily in the first chunks.
    fill_q = deque()
    fill_budget = [10 ** 9]

    def fill(n=1):
        for _ in range(n):
            if not fill_q or fill_budget[0] <= 0:
                return
            fill_budget[0] -= 1
            fill_q.popleft()()

    def fill_pe(k=1):
        # emit closures until k PE-heavy ones landed: latency-critical call
        # sites need actual matmul work between two dependent PE ops, not a
        # DMA/elementwise closure
        while k > 0 and fill_q and fill_budget[0] > 0:
            fill_budget[0] -= 1
            cl = fill_q.popleft()
            cl()
            if getattr(cl, "has_pe", False):
                k -= 1

    # P[s % 2]: per-strip proj outputs (tiles), written by filler closures
    P = [dict(), dict()]

    def make_xdma(s):
        def xdma():
            st = P[s % 2]
            l0 = s * LT
            xr = x.rearrange("(t p) l -> p t l", p=128)
            for kt8 in range(8):
                xk = xpool.tile([128, LT], BF, tag=f"xt{kt8}", name=f"xt{kt8}")
                nc.sync.dma_start(out=xk[:], in_=xr[:, kt8, l0:l0 + LT])
                st[("x", kt8)] = xk
            betas = small.tile([128, CPS, 2], FP, tag="betas", name="betas")
            nc.sync.dma_start(out=betas[:], in_=bt[:, s])
            st["betas"] = betas
        return xdma

    # each projection unit is split in two closures (uA: projection matmuls
    # + conv-window copy; uB: conv matmuls + silu + squares) queued
    # staggered, so uA(i+1)'s matmuls hide uA(i)'s Act-engine copy and the
    # conv matmuls never head-of-line block the PE on the ze handoff.
    def make_unit_a(s, name, ct):
        def unit_a():
            st = P[s % 2]
            zp = psA.tile([128, LT], FP, tag="mm512", name="zp")
            for kt8 in range(8):
                nc.tensor.matmul(zp[:], wt[(name, kt8)][:, bass.ts(ct, 128)],
                                 st[("x", kt8)][:],
                                 start=(kt8 == 0), stop=(kt8 == 7))
            ze = zpool.tile([128, KT - 1 + LT], BF, tag="zext", name="ze")
            nc.gpsimd.tensor_copy(ze[:, 0:KT - 1], carries[(name, ct)][:])
            nc.scalar.copy(ze[:, KT - 1:], zp[:])
            nc.gpsimd.tensor_copy(carries[(name, ct)][:],
                                  ze[:, LT:LT + KT - 1])
            st[("ze", name, ct)] = ze
        unit_a.has_pe = True
        return unit_a

    def make_unit_b(s, name, ct):
        def unit_b():
            st = P[s % 2]
            ze = st.pop(("ze", name, ct))
            zc = psA.tile([128, LT], FP, tag="mm512", name="zc")
            for j in range(KT):
                nc.tensor.matmul(zc[:], diag[name][:, ct, j, :],
                                 ze[:, j:j + LT],
                                 start=(j == 0), stop=(j == KT - 1))
            if name == "v":
                t = stripD.tile([128, LT], BF, tag=f"zs_v{ct}", name="zsv")
            else:
                t = stripT.tile([128, LT], BF, tag=f"zs_{name}{ct}", name="zs")
            nc.scalar.activation(t[:], zc[:], AF.Silu)
            st[(name, ct)] = t
            if name in ("q", "k"):
                sqt = stripT.tile([128, LT], BF, tag=f"sq_{name}{ct}",
                                  name="sqt")
                if name == "q":
                    nc.vector.tensor_tensor(sqt[:], t[:], t[:], op=OP.mult)
                else:
                    nc.gpsimd.tensor_mul(sqt[:], t[:], t[:])
                st[("sq", name, ct)] = sqt
        unit_b.has_pe = True
        return unit_b

    def make_norms(s):
        # rsqall column layout: col = h*12 + lt*3 + r with r in
        # {0: rsq_q, 1: rsq_k, 2: rsq_k*beta} so one strided broadcast DMA
        # per head can build the [ch, lt, r, tok] bc tile directly.
        def norms():
            st = P[s % 2]
            # shares the projection pool's banks: the l2 colsums are one
            # brief use per strip, freeing a 6th bank for the scan rotation
            psq = psA.tile([128, 512], FP, tag="mm512", name="psq")
            for h in range(2):
                for lt in range(CPS):
                    base = h * 12 + lt * 3
                    # accumulation groups must not interleave within a PSUM
                    # bank: a start=True re-marks the whole 2KB zero region,
                    # so an open group's accumulate step would read zeros
                    for ct2 in range(2):
                        nc.tensor.matmul(
                            psq[:, base:base + 1],
                            st[("sq", "q", 2 * h + ct2)][:, bass.ts(lt, 128)],
                            ones_col[:], start=(ct2 == 0), stop=(ct2 == 1))
                    # k colsum lands in both r=1 and r=2 so the quake pass
                    # runs on one contiguous [128, 24] block
                    for ct2 in range(2):
                        nc.tensor.matmul(
                            psq[:, base + 1:base + 3],
                            st[("sq", "k", 2 * h + ct2)][:, bass.ts(lt, 128)],
                            ones2[:], start=(ct2 == 0), stop=(ct2 == 1))
            rsqall = small.tile([128, 24], BF, tag="rsqall", name="rsqall")
            dve_rsqrt(24, rsqall[:], psq[:, 0:24], 1.0,
                      EPS_L2 * EPS_L2, "rsA", iters=2)
            st["rsqall"] = rsqall
        return norms

    def make_g(s, lt):
        def g_unit():
            st = P[s % 2]
            if lt == 0:
                st["gg"] = stripD.tile([128, CPS, 512], BF, tag="gg",
                                       name="gg")
            gp = psA.tile([128, 512], FP, tag="mm512", name="gp")
            for kt8 in range(8):
                nc.tensor.matmul(gp[:], st[("x", kt8)][:, bass.ts(lt, 128)],
                                 wt[("g", kt8)][:],
                                 start=(kt8 == 0), stop=(kt8 == 7))
            # the gate g*sigmoid(g) is exactly silu(g)
            nc.scalar.activation(st["gg"][:, lt, :], gp[:], AF.Silu)
        g_unit.has_pe = True
        return g_unit

    def make_tail0(s):
        def tail0():
            st = P[s % 2]
            rsqall = st["rsqall"]
            rsv = rsqall[:].rearrange("p (h l r) -> p h l r", h=2, r=3)
            nc.vector.tensor_tensor(
                rsv[:, :, :, 2], rsv[:, :, :, 1],
                st["betas"][:].rearrange("p l h -> p h l"), op=OP.mult)
            st["bc"] = {}
            # transpose on PE (bf16, 128 rows), then broadcast across the
            # 128 channel partitions with a DRAM bounce: contiguous copy
            # out, partition-stride-0 read back (SBUF DMA APs cannot cross
            # partitions and are capped at 3 dims).
            rpt = psS.tile([128, 512], BF, tag="scan", name="rpt")
            nc.tensor.matmul(rpt[0:24, 0:128], rsqall[:], ident_b[:],
                             is_transpose=True, start=True, stop=True)
            rr24 = small.tile([24, 128], BF, tag="rr24", name="rr24")
            nc.scalar.copy(rr24[:], rpt[0:24, 0:128])
            nc.scalar.dma_start(out=bcd[s % 2], in_=rr24[:])
            for h in range(2):
                bc = stripT.tile([128, CPS, 3, 128], BF, tag=f"bcast{h}",
                                 name="bc")
                nc.scalar.dma_start(
                    out=bc[:],
                    in_=bcd[s % 2, h * 12:(h + 1) * 12].rearrange(
                        "a b -> (a b)").partition_broadcast(128))
                st["bc"][h] = bc
            st["kqT"], st["khT"] = {}, {}
        return tail0

    def make_tail_h(s, h, ct2):
        def tail_h():
            st = P[s % 2]
            bc = st["bc"][h]
            ct = 2 * h + ct2

            def c4(ap):
                return ap.rearrange("p (a b) -> p a b", a=CPS)

            kq = stripD.tile([128, 2, LT], BF, tag=f"kqT{h}{ct2}",
                             name="kq")
            # strips 0 / NS-1 have no scan slack around their boundary:
            # route the kb product to DVE so the Pool queue isn't serial
            if s == 0 or s == NS - 1:
                nc.vector.tensor_tensor(c4(kq[:, 0, :]), c4(st[("k", ct)][:]),
                                        bc[:, :, 2, :], op=OP.mult)
            else:
                nc.gpsimd.tensor_mul(c4(kq[:, 0, :]), c4(st[("k", ct)][:]),
                                     bc[:, :, 2, :])
            nc.vector.tensor_tensor(c4(kq[:, 1, :]), c4(st[("q", ct)][:]),
                                    bc[:, :, 0, :], op=OP.mult)
            kh = stripD.tile([128, LT], BF, tag=f"khT{h}{ct2}", name="kh")
            nc.gpsimd.tensor_mul(c4(kh[:]), c4(st[("k", ct)][:]),
                                 bc[:, :, 1, :])
            st["kqT"][(h, ct2)] = kq
            st["khT"][(h, ct2)] = kh
        return tail_h

    def push_strip(s, xdma=True):
        if xdma:
            fill_q.append(make_xdma(s))
        else:
            P[s % 2].update(x0_tiles)
            P[s % 2]["betas"] = bt0
        # qkv units staggered in halves: uA(i+1) queued between uA(i) and
        # uB(i) so the PE always has projection matmuls while the Act-engine
        # ze copies land.
        units = [(n, ct) for n in ("q", "k") for ct in range(4)]
        prev = None
        for n, ct in units:
            fill_q.append(make_unit_a(s, n, ct))
            if prev is not None:
                fill_q.append(make_unit_b(s, *prev))
            prev = (n, ct)
        fill_q.append(make_unit_a(s, "v", 0))
        fill_q.append(make_unit_b(s, *prev))
        fill_q.append(make_norms(s))
        # interleave the (DVE/DMA-heavy) tail closures with the (PE-heavy)
        # v units so no engine sees a multi-us lump and the tail0 PE
        # transpose never queues directly behind a long DVE backlog; the
        # reserved last-two closures are g units, which nothing in the next
        # scan's first chunk depends on
        fill_q.append(make_unit_a(s, "v", 1))
        fill_q.append(make_unit_b(s, "v", 0))
        fill_q.append(make_tail0(s))
        vq = [("v", 2), ("v", 3)]
        seq = []
        for i, (h, ct2) in enumerate(((0, 0), (0, 1), (1, 0), (1, 1))):
            if i < 2:
                seq.append(make_unit_a(s, *vq[i]))
                seq.append(make_unit_b(s, "v", i + 1))
            elif i == 2:
                seq.append(make_unit_b(s, "v", 3))
                seq.append(make_g(s, 0))
            else:
                seq.append(make_g(s, 1))
            seq.append(make_tail_h(s, h, ct2))
        fill_q.extend(seq)
        for lt in range(2, CPS):
            fill_q.append(make_g(s, lt))

    # ---- scan ---------------------------------------------------------------
    # output flushes run TWO chunks behind the scan: the DVE rsqrt/gate
    # chain of a strip's last chunk then hides behind the next strip's
    # first-chunk scan instead of stalling the PE at the boundary.
    pending = deque()

    def flush_pending(force=False, limit=None):
        n = 0
        while len(pending) >= (1 if force else 2):
            flush_one()
            n += 1
            if limit is not None and n >= limit:
                return

    def flush_one():
        plt, pl0, pogh = pending.popleft()
        # ogh was computed eagerly a chunk ago, so the PE transposes here
        # never wait on the DVE rsqrt/gate chain.
        otT = small.tile([128, 4, 128], BF, tag="otT")
        for h in range(2):
            for q2 in range(2):
                q4 = 2 * h + q2
                tp = psS.tile([128, 512], BF, tag="scan", name="tpf")
                nc.tensor.matmul(tp[:, 0:128], pogh[h][:, bass.ts(q2, 128)],
                                 ident_b[:], is_transpose=True,
                                 start=True, stop=True)
                if q4 % 2 == 0:
                    nc.scalar.copy(otT[:, q4, :], tp[:, 0:128])
                else:
                    nc.vector.tensor_copy(otT[:, q4, :], tp[:, 0:128])
        for nh in range(2):
            pop = psA.tile([128, 512], FP, tag="mm512", name="pop")
            for q4 in range(4):
                nc.tensor.matmul(pop[:], otT[:, q4, :],
                                 wo_t[:, q4, bass.ts(nh, 512)],
                                 start=(q4 == 0), stop=(q4 == 3))
            ou = small.tile([128, 512], FP, tag="outsb", name="ou")
            nc.scalar.copy(ou[:], pop[:])
            nc.sync.dma_start(
                out[pl0 + plt * 128:pl0 + (plt + 1) * 128,
                    nh * 512:(nh + 1) * 512], ou[:])

    Tout = [None, None]

    def scan_T(s, lt):
        # S-independent path for chunk lt: transposes, A products, masks,
        # Neumann inverse.  Emitted one chunk AHEAD so its cross-engine
        # handoffs hide behind the previous chunk's S path and fillers.
        st = P[s % 2]
        betas = st["betas"]
        kqT, khT = st["kqT"], st["khT"]
        cs = bass.ts(lt, 128)
        ktk, vb, ATat, Alow = {}, {}, {}, {}
        R, Lk, Uk = {}, {}, {}
        p1, p2, pp, pw = {}, {}, {}, {}
        for h in range(2):
            ktk[h] = hot.tile([128, 2, 128], BF, tag="ktok", name=f"ktk{h}")
            vb[h] = hot.tile([128, 256], BF, tag="vb", name=f"vb{h}")
            bcol = betas[:, lt, h:h + 1]
            for ct2 in range(2):
                tpk = psS.tile([128, 512], BF, tag="scan", name="tpk")
                nc.tensor.matmul(tpk[:, 0:128], khT[(h, ct2)][:, cs],
                                 ident_b[:], is_transpose=True,
                                 start=True, stop=True)
                tpv = psS.tile([128, 512], BF, tag="scan", name="tpv")
                nc.tensor.matmul(tpv[:, 0:128],
                                 st[("v", 2 * h + ct2)][:, cs],
                                 ident_b[:], is_transpose=True,
                                 start=True, stop=True)
                nc.vector.tensor_copy(ktk[h][:, ct2, :], tpk[:, 0:128])
                nc.scalar.mul(vb[h][:, bass.ts(ct2, 128)], tpv[:, 0:128],
                              bcol)
        for h in range(2):
            p1[h] = psS.tile([128, 512], FP, tag="scan", name="p1")
            for ct2 in range(2):
                nc.tensor.matmul(p1[h][:, 0:256], khT[(h, ct2)][:, cs],
                                 kqT[(h, ct2)][:, :, cs],
                                 start=(ct2 == 0), stop=(ct2 == 1))
            p2[h] = psS.tile([128, 512], FP, tag="scan", name="p2")
            for ct2 in range(2):
                nc.tensor.matmul(p2[h][:, 0:128], kqT[(h, ct2)][:, 0, cs],
                                 khT[(h, ct2)][:, cs],
                                 start=(ct2 == 0), stop=(ct2 == 1))
        for h in range(2):
            ATat[h] = hot.tile([128, 256], BF, tag="ATat", name=f"ATat{h}")
            nc.vector.tensor_tensor(ATat[h][:], p1[h][:, 0:256],
                                    mask_ua.rearrange("p a b -> p (a b)"),
                                    op=OP.mult)
            Alow[h] = hot.tile([128, 128], BF, tag="Alow", name=f"Alow{h}")
            nc.vector.tensor_tensor(Alow[h][:], p2[h][:, 0:128],
                                    mask_sl[:], op=OP.mult)
            R[h] = hot.tile([128, 128], BF, tag="Rn", name=f"R{h}")
            nc.vector.tensor_tensor(R[h][:], ident_b[:], ATat[h][:, 0:128],
                                    op=OP.subtract)
            Lk[h] = Alow[h][:, 0:128]
            Uk[h] = ATat[h][:, 0:128]
        fill_pe(2)
        # Neumann: (I - A_T)(I + A_T^2)(I + A_T^4), squarings and lagged
        # R-updates in one psum tile/copy per level per head
        for lev in range(NLEV):
            for h in range(2):
                pp[h] = psS.tile([128, 512], FP, tag="scan", name="pp")
                nc.tensor.matmul(pp[h][:, 0:128], Uk[h], Lk[h],
                                 start=True, stop=True)
                nc.tensor.matmul(pp[h][:, 128:256], Lk[h], Uk[h],
                                 start=True, stop=True)
                if lev > 0:
                    nc.tensor.matmul(pp[h][:, 256:384], Lk[h], R[h][:],
                                     start=True, stop=True)
            for h in range(2):
                LUR = hot.tile([128, 384], BF, tag="LUR", name=f"LUR{h}")
                if (lev + h) % 2 == 0:
                    nc.scalar.copy(LUR[:, 0:256], pp[h][:, 0:256])
                else:
                    nc.vector.tensor_copy(LUR[:, 0:256], pp[h][:, 0:256])
                if lev > 0:
                    # R <- A^{2^lev} R + R: add the old R while draining psum
                    nc.vector.scalar_tensor_tensor(
                        LUR[:, 256:384], pp[h][:, 256:384], 1.0, R[h][:],
                        op0=OP.mult, op1=OP.add)
                    R[h] = LUR[:, 256:384]
                Lk[h] = LUR[:, 0:128]
                Uk[h] = LUR[:, 128:256]
            fill_pe(1)
        Rf = {}
        for h in range(2):
            pw[h] = psS.tile([128, 512], FP, tag="scan", name="pw")
            # final factor: R <- (I + A_T^{2^NLEV}) R
            nc.tensor.matmul(pw[h][:, 256:384], Lk[h], R[h][:],
                             start=True, stop=True)
        for h in range(2):
            Rf[h] = hot.tile([128, 128], BF, tag="Rf", name=f"Rf{h}")
            nc.vector.scalar_tensor_tensor(Rf[h][:], pw[h][:, 256:384], 1.0,
                                           R[h][:], op0=OP.mult, op1=OP.add)
        fill()
        Tout[lt % 2] = (ktk, vb, ATat, Rf)

    def scan_S(s, lt):
        # S-dependent path: kb@S residual, U = T(vb - kb S), outputs and
        # the state update.  Consumes Tout (whose T path was emitted a
        # chunk ahead, so its cross-engine chain is already drained).
        st = P[s % 2]
        l0 = s * LT
        kqT = st["kqT"]
        ktk, vb, ATat, Rf = Tout[lt % 2]
        cs = bass.ts(lt, 128)
        pkS, rhs2 = {}, {}
        for h in range(2):
            pkS[h] = psS.tile([128, 512], FP, tag="scan", name="pkS")
            for ct2 in range(2):
                nc.tensor.matmul(pkS[h][:, 0:256], kqT[(h, ct2)][:, 0, cs],
                                 Sbf[h][:, ct2, :],
                                 start=(ct2 == 0), stop=(ct2 == 1))
        for h in range(2):
            rhs2[h] = hot.tile([128, 256], BF, tag="rhs2", name=f"rhs2{h}")
            nc.vector.tensor_tensor(rhs2[h][:], vb[h][:], pkS[h][:, 0:256],
                                    op=OP.subtract)
        fill(1)
        ssq2 = small.tile([128, 2], FP, tag="ssq_o", name="ssq2")
        Ut, pos, pu = {}, {}, {}
        for h in range(2):
            # U = T (vb - kb S), single matmul off the bf16 residual
            pu[h] = psS.tile([128, 512], FP, tag="scan", name="pu")
            nc.tensor.matmul(pu[h][:, 0:256], Rf[h][:], rhs2[h][:],
                             start=True, stop=True)
        for h in range(2):
            Ut[h] = hot.tile([128, 256], BF, tag="Ut", name=f"Ut{h}")
            if h == 0:
                nc.vector.tensor_copy(Ut[h][:], pu[h][:, 0:256])
            else:
                nc.scalar.copy(Ut[h][:], pu[h][:, 0:256])
        for h in range(2):
            po = psS.tile([128, 512], FP, tag="scan", name="po")
            for half in range(2):
                nc.tensor.matmul(po[:, 0:256], kqT[(h, half)][:, 1, cs],
                                 Sbf[h][:, half, :],
                                 start=(half == 0), stop=False)
            nc.tensor.matmul(po[:, 0:256], ATat[h][:, 128:256], Ut[h][:],
                             start=False, stop=True)
            pos[h] = po
            psu = psS.tile([128, 512], FP, tag="scan", name="psu")
            for half in range(2):
                nc.tensor.matmul(psu[:, bass.ts(half, 256)],
                                 ktk[h][:, half, :], Ut[h][:],
                                 start=True, stop=True)
            g = s * CPS + lt
            s_in = S32[h][g % 2][:].rearrange("p a b -> p (a b)")
            s_out = S32[h][1 - g % 2][:].rearrange("p a b -> p (a b)")
            nc.vector.tensor_tensor(s_out, psu[:, 0:512], s_in, op=OP.add)
            for half in range(2):
                nc.gpsimd.tensor_copy(Sbf[h][:, half, :],
                                      S32[h][1 - g % 2][:, half, :])
            scrd = hot.tile([128, 256], BF, tag="scrd", name="scrd")
            nc.scalar.activation(scrd[:], po[:, 0:256], AF.Square,
                                 accum_out=ssq2[:, h:h + 1])
        fill()

        # gate compute (rsqrt chain + gate multiply) emitted EAGERLY so the
        # deferred flush's PE transposes find ogh ready; only the PE/DMA
        # tail (transpose + out-proj + store) is deferred a chunk.
        rv2 = small.tile([128, 2], FP, tag="rv_o", name="rv2")
        dve_rsqrt(2, rv2[:], ssq2[:], 1.0 / Dh, EPS_RMS, "rvq", iters=1)
        oghs = {}
        for h in range(2):
            ogh = hot.tile([128, 256], BF, tag="ogh", name=f"ogh{h}",
                           bufs=16)
            nc.vector.scalar_tensor_tensor(
                ogh[:], pos[h][:, 0:256], rv2[:, h:h + 1],
                st["gg"][:, lt, bass.ts(h, 256)], op0=OP.mult, op1=OP.mult)
            oghs[h] = ogh
        pending.append((lt, l0, oghs))

    # ---- main loop ----------------------------------------------------------
    # scan_T for chunk g+1 is emitted BEFORE scan_S for chunk g, so chunk
    # g's S-chain (pkS -> rhs2 -> pu -> ... -> Sbf) overlaps chunk g+1's
    # T-chain (transposes, A products, Neumann) instead of serializing.
    push_strip(0, xdma=False)
    fill(100)
    TOT = NS * CPS
    scan_T(0, 0)
    for s in range(NS):
        n_left = len(fill_q)  # strip s closures reserved past its proj phase
        if s + 1 < NS:
            push_strip(s + 1)
        for lt in range(CPS):
            g = s * CPS + lt
            if lt == 0:
                fill_budget[0] = n_left
                fill(n_left)
            # strip NS-2 banks its output flushes; strip NS-1 drains the
            # backlog two per chunk so the pipeline tail has PE work
            if s == NS - 2:
                pass
            elif s == NS - 1:
                flush_pending(force=True, limit=2)
            else:
                flush_pending()
            rem = len(fill_q) - (2 if s + 1 < NS else 0)
            # next strip's proj/tail closures must all be emitted before
            # chunk (s,3) emits scan_T(s+1,0): spread them over chunks 0-2
            den = max(1, CPS - 1 - lt) if s + 1 < NS else max(1, CPS - lt)
            fill_budget[0] = max(0, -(-rem // den))
            if g + 1 < TOT:
                if (g + 1) % CPS == 0:
                    # entering strip s+1: drain everything its chunk-0 T
                    # path needs (the 2 reserved g-units may stay queued)
                    fill_budget[0] = max(0, len(fill_q) - 2)
                    fill(fill_budget[0])
                scan_T((g + 1) // CPS, (g + 1) % CPS)
            scan_S(s, lt)
    flush_pending(force=True)


_CACHED_NC = None


def _build():
    global _CACHED_NC
    if _CACHED_NC is not None:
        return _CACHED_NC
    nc = bacc.Bacc("TRN2", target_bir_lowering=False, debug=False)
    io = {}
    io["x"] = nc.dram_tensor("x", [D, L], BF, kind="ExternalInput").ap()
    for nm, shp in (("wq", [D, DL]), ("wk", [D, DL]), ("wv", [D, DL]),
                    ("wg", [D, DL])):
        io[nm] = nc.dram_tensor(nm, shp, BF, kind="ExternalInput").ap()
    io["bt"] = nc.dram_tensor("bt", [128, NS, CPS, 2], FP,
                              kind="ExternalInput").ap()
    io["bcd"] = nc.dram_tensor("bcd", [2, 24, 128], BF, kind="Internal").ap()
    io["wo"] = nc.dram_tensor("wo", [DL, D], BF, kind="ExternalInput").ap()
    for nm in ("cq", "ck", "cv"):
        io[nm] = nc.dram_tensor(nm, [DL, KT], FP, kind="ExternalInput").ap()
    io["out"] = nc.dram_tensor("out", [L, D], FP, kind="ExternalOutput").ap()
    with tile.TileContext(nc) as tc, ExitStack() as ctx:
        deltanet_core(ctx, tc, io)
    nc.compile()
    _CACHED_NC = nc
    return nc


def kernel(hidden_states, Wq, Wk, Wv, Wb, Wg, Wo, conv_q, conv_k, conv_v,
           norm_w):
    import ml_dtypes
    bf = ml_dtypes.bfloat16
    x = np.ascontiguousarray(np.asarray(hidden_states, dtype=np.float32))
    Wo_s = np.asarray(Wo, np.float32) * np.tile(np.asarray(norm_w, np.float32),
                                                H)[:, None]
    Wb_f = np.asarray(Wb, np.float32)
    nc = _build()
    in_maps = []
    for c in range(8):
        b, hg = c // 2, c % 2
        cols = slice(hg * DL, (hg + 1) * DL)
        # beta = sigmoid(x @ Wb) is input-only; fold it on the host like
        # the norm_w scaling of Wo.  [L, 2] -> [128, NS, CPS, 2]
        logits = x[b] @ Wb_f[:, 2 * hg:2 * hg + 2]
        beta = 1.0 / (1.0 + np.exp(-logits))
        bt = beta.reshape(NS, CPS, 128, 2).transpose(2, 0, 1, 3)
        in_maps.append({
            "x": np.ascontiguousarray(x[b].T.astype(bf)),
            "wq": np.ascontiguousarray(np.asarray(Wq, np.float32)[:, cols].astype(bf)),
            "wk": np.ascontiguousarray(np.asarray(Wk, np.float32)[:, cols].astype(bf)),
            "wv": np.ascontiguousarray(np.asarray(Wv, np.float32)[:, cols].astype(bf)),
            "wg": np.ascontiguousarray(np.asarray(Wg, np.float32)[:, cols].astype(bf)),
            "bt": np.ascontiguousarray(bt),
            "wo": np.ascontiguousarray(Wo_s[cols, :].astype(bf)),
            "cq": np.ascontiguousarray(np.asarray(conv_q, np.float32)[cols]),
            "ck": np.ascontiguousarray(np.asarray(conv_k, np.float32)[cols]),
            "cv": np.ascontiguousarray(np.asarray(conv_v, np.float32)[cols]),
        })
    res = run_bass_kernel_spmd(nc, in_maps, core_ids=list(range(8)))
    outv = np.zeros((B, L, D), np.float32)
    for c in range(8):
        outv[c // 2] += res.results[c]["out"]
    return outv



# revision 43
# speedup vs baseline: 1.0169x; 1.0169x over previous
"""DeltaNet forward Trainium2 kernel (8-core SPMD, batch x head-pair sharded).

Sharding: core c -> batch b=c//2, head-group hg=c%2 (heads 2hg, 2hg+1 = columns
hg*512 : hg*512+512 of the q/k/v/o projections).  Each core computes a partial
[L, D] output (its two heads' contribution through the output projection); the
host sums the two partials per batch.  norm_w is folded into Wo on the host.

Per-core pipeline, software-pipelined at emission level: the projection work
for strip s+1 is queued as ~20 filler closures and drained at designated
stall points inside strip s's scan chunks, so the (in-order) PE stream has
gap-filler matmuls wherever the scan's cross-engine handoffs would stall it.

  x^T strip (DMA xbar-transposed load, bf16)
  -> q/k/v projections channel-major z^T = W^T x^T (bf16 matmuls)
  -> depthwise causal conv = 4 accumulating diag(w_tap) matmuls (bf16)
  -> silu as z * sigmoid(z): sigmoid on Act engine, multiply on Pool engine
     (keeps the Act engine on a single act-function table all kernel long)
  -> l2norm scales via ones-matmul column sums; rsqrt via DVE quake+Newton
  -> per 128-token chunk: delta-rule scan in bf16; (I+A)^-1 by Neumann product
     form; S in fp32 master + bf16 working copy
  -> RMS-norm (DVE rsqrt) + swish gate, output projection (fp32r)
"""

import sys

sys.path.insert(0, "/opt/trn_rl_repo")

from collections import deque
from contextlib import ExitStack

import numpy as np

import concourse.bass as bass
import concourse.tile as tile
from concourse import bacc, mybir
from concourse.bass_utils import run_bass_kernel_spmd

FP = mybir.dt.float32
FR = mybir.dt.float32r
BF = mybir.dt.bfloat16
I32 = mybir.dt.int32
AF = mybir.ActivationFunctionType
OP = mybir.AluOpType

B, L, D, H = 4, 4096, 1024, 4
Dh = 256          # head dim
DL = 512          # per-core channels (2 heads)
KT = 4            # conv taps
C = 128           # chunk length (our choice; the math is chunk-size invariant)
LT = 512          # L-strip size
NS = L // LT      # 8 strips
CPS = LT // C     # 4 chunks per strip
NLEV = 2          # Neumann levels: exact would be 6 (A^64); A is
                  # strongly contractive here (||A||~0.5), A^8 ~ 0
EPS_RMS = 1e-5
EPS_L2 = 1e-12
QK3 = 0x5F3759DF  # quake fast-rsqrt seed constant


def deltanet_core(ctx: ExitStack, tc: tile.TileContext, io: dict):
    nc = tc.nc
    x, wq, wk, wv, wg, bt, wo, cq, ck, cv, out = (
        io["x"], io["wq"], io["wk"], io["wv"], io["wg"], io["bt"], io["wo"],
        io["cq"], io["ck"], io["cv"], io["out"])
    bcd = io["bcd"]

    pool1 = ctx.enter_context(tc.tile_pool(name="consts", bufs=1))
    xpool = ctx.enter_context(tc.tile_pool(name="xp", bufs=2))
    zpool = ctx.enter_context(tc.tile_pool(name="zp", bufs=4))
    stripT = ctx.enter_context(tc.tile_pool(name="stripT", bufs=1))
    stripD = ctx.enter_context(tc.tile_pool(name="stripD", bufs=2))
    small = ctx.enter_context(tc.tile_pool(name="small", bufs=2))
    hot = ctx.enter_context(tc.tile_pool(name="hot", bufs=5))
    spool = ctx.enter_context(tc.tile_pool(name="state", bufs=1))
    psA = ctx.enter_context(tc.tile_pool(name="psA", bufs=2, space="PSUM"))
    psS = ctx.enter_context(tc.tile_pool(name="psS", bufs=6, space="PSUM"))

    # ---- resident constants -------------------------------------------------
    # wq first, then strip 0's x, so the first projection can start ~6us in
    # instead of waiting for the whole constant working set to land.
    wt = {}
    x0_tiles = {}
    for name, w in (("q", wq),):
        wr = w.rearrange("(t p) n -> p t n", p=128)
        for kt8 in range(8):
            t = pool1.tile([128, 512], BF, tag=f"w{name}{kt8}",
                           name=f"w{name}{kt8}")
            nc.sync.dma_start(t[:], wr[:, kt8, :])
            wt[(name, kt8)] = t
    xr0 = x.rearrange("(t p) l -> p t l", p=128)
    for kt8 in range(8):
        xk = xpool.tile([128, LT], BF, tag=f"xt{kt8}", name=f"xt{kt8}")
        # Act DMA ring: runs concurrently with the weight loads on SP's ring
        nc.scalar.dma_start(out=xk[:], in_=xr0[:, kt8, 0:LT])
        x0_tiles[("x", kt8)] = xk
    cw = {}
    for name, cz in (("q", cq), ("k", ck), ("v", cv)):
        t = pool1.tile([128, 4, KT], FP, tag=f"c{name}")
        nc.sync.dma_start(t[:], cz.rearrange("(t p) j -> p t j", p=128))
        cw[name] = t
    bt0 = pool1.tile([128, CPS, 2], FP, tag="bt0")
    nc.sync.dma_start(bt0[:], bt[:, 0])
    for name, w in (("k", wk), ("v", wv), ("g", wg)):
        wr = w.rearrange("(t p) n -> p t n", p=128)
        for kt8 in range(8):
            t = pool1.tile([128, 512], BF, tag=f"w{name}{kt8}",
                           name=f"w{name}{kt8}")
            nc.sync.dma_start(t[:], wr[:, kt8, :])
            wt[(name, kt8)] = t
    wo_t = pool1.tile([128, 4, 1024], BF, tag="wo")
    nc.sync.dma_start(wo_t[:], wo.rearrange("(t p) n -> p t n", p=128))

    # mask_ua[:, 0, :] strict-upper, mask_ua[:, 1, :] inclusive-upper:
    # keep where x - a - y < 0  (a=0 -> x<y, a=1 -> x<=y)
    mask_ua = pool1.tile([128, 2, 128], FP, tag="mask_ua")
    nc.gpsimd.memset(mask_ua, 1.0)
    nc.gpsimd.affine_select(out=mask_ua[:], in_=mask_ua[:],
                            compare_op=OP.is_gt, fill=0.0, base=0,
                            pattern=[[1, 2], [1, 128]], channel_multiplier=-1)
    mask_sl = pool1.tile([128, 128], FP, tag="mask_sl")
    nc.gpsimd.memset(mask_sl, 1.0)
    nc.gpsimd.affine_select(out=mask_sl[:], in_=mask_sl[:],
                            compare_op=OP.is_gt, fill=0.0, base=0,
                            pattern=[[-1, 128]], channel_multiplier=1)
    from concourse.masks import make_identity
    ident_b = pool1.tile([128, 128], BF, tag="identb")
    make_identity(nc, ident_b)

    ones_col = pool1.tile([128, 1], BF, tag="ones_col")
    nc.vector.memset(ones_col, 1.0)
    ones2 = pool1.tile([128, 2], BF, tag="ones2")
    nc.vector.memset(ones2, 1.0)

    # conv diag tiles (bf16): diag(w_tap) per (tensor, ch-tile, tap).
    diag = {}
    for name in ("q", "k", "v"):
        dt = pool1.tile([128, 4, KT, 128], BF, tag=f"diag{name}")
        diag[name] = dt
        for ct in range(4):
            for j in range(KT):
                nc.vector.tensor_scalar_mul(dt[:, ct, j, :], ident_b[:],
                                            cw[name][:, ct, j:j + 1])

    # ---- persistent state ---------------------------------------------------
    # S32 ping-pongs between two buffers per head: the update writes the
    # other buffer (reading the old one), so chunk g+1's update never WARs
    # against chunk g's Sbf drain and the S critical chain is add-to-add.
    S32 = [[spool.tile([128, 2, 256], FP, name=f"S32_{h}{b}",
                       tag=f"S32_{h}{b}") for b in range(2)]
           for h in range(2)]
    Sbf = [spool.tile([128, 2, 256], BF, name=f"Sbf_{h}", tag=f"Sbf_{h}")
           for h in range(2)]
    for h in range(2):
        nc.vector.memset(S32[h][0], 0.0)
        nc.vector.memset(Sbf[h], 0.0)
    carries = {}
    for name in ("q", "k", "v"):
        for ct in range(4):
            cr = spool.tile([128, KT - 1], BF, tag=f"carry_{name}{ct}")
            nc.vector.memset(cr, 0.0)
            carries[(name, ct)] = cr

    # ---- DVE fast rsqrt (quake seed + Newton) ------------------------------
    # out = 1/sqrt(in*scale + bias); `inp` may live in PSUM.  Keeps the Act
    # engine free of Abs_reciprocal_sqrt so one act table serves the kernel.
    def dve_rsqrt(n, out_ap, inp, scale, bias, tag, iters, eng=None):
        # eng=nc.gpsimd runs the whole chain on Pool (inp must be SBUF)
        eng = eng or nc.vector
        xb = small.tile([128, n], FP, tag=f"{tag}_x", name=f"{tag}_x")
        eng.tensor_scalar(xb[:], inp, scale, bias,
                          op0=OP.mult, op1=OP.add)
        yb = small.tile([128, n], FP, tag=f"{tag}_y", name=f"{tag}_y")
        eng.tensor_scalar(yb.bitcast(I32)[:], xb.bitcast(I32)[:], 1,
                          None, op0=OP.logical_shift_right)
        eng.tensor_scalar(yb.bitcast(I32)[:], yb.bitcast(I32)[:], -1,
                          QK3, op0=OP.mult, op1=OP.add)
        for it in range(iters):
            hb = small.tile([128, n], FP, tag=f"{tag}_h{it}",
                            name=f"{tag}_h{it}")
            eng.tensor_tensor(hb[:], yb[:], yb[:], op=OP.mult)
            eng.tensor_tensor(hb[:], hb[:], xb[:], op=OP.mult)
            eng.tensor_scalar(hb[:], hb[:], -0.5, 1.5,
                              op0=OP.mult, op1=OP.add)
            if it == iters - 1:
                eng.tensor_tensor(out_ap, yb[:], hb[:], op=OP.mult)
            else:
                y2 = small.tile([128, n], FP, tag=f"{tag}_y{it}",
                                name=f"{tag}_y{it}")
                eng.tensor_tensor(y2[:], yb[:], hb[:], op=OP.mult)
                yb = y2

    # ---- filler queue: strip s+1 proj work drained inside strip s scan -----
    # Metered per chunk so chunk 3 (and the strip boundary) still has filler
    # matmuls instead of the queue draining greedily in the first chunks.
    fill_q = deque()
    fill_budget = [10 ** 9]

    def fill(n=1):
        for _ in range(n):
            if not fill_q or fill_budget[0] <= 0:
                return
            fill_budget[0] -= 1
            fill_q.popleft()()

    def fill_pe(k=1):
        # emit closures until k PE-heavy ones landed: latency-critical call
        # sites need actual matmul work between two dependent PE ops, not a
        # DMA/elementwise closure
        while k > 0 and fill_q and fill_budget[0] > 0:
            fill_budget[0] -= 1
            cl = fill_q.popleft()
            cl()
            if getattr(cl, "has_pe", False):
                k -= 1

    # P[s % 2]: per-strip proj outputs (tiles), written by filler closures
    P = [dict(), dict()]

    def make_xdma(s):
        def xdma():
            st = P[s % 2]
            l0 = s * LT
            xr = x.rearrange("(t p) l -> p t l", p=128)
            for kt8 in range(8):
                xk = xpool.tile([128, LT], BF, tag=f"xt{kt8}", name=f"xt{kt8}")
                nc.sync.dma_start(out=xk[:], in_=xr[:, kt8, l0:l0 + LT])
                st[("x", kt8)] = xk
            betas = small.tile([128, CPS, 2], FP, tag="betas", name="betas")
            nc.sync.dma_start(out=betas[:], in_=bt[:, s])
            st["betas"] = betas
        return xdma

    # each projection unit is split in two closures (uA: projection matmuls
    # + conv-window copy; uB: conv matmuls + silu + squares) queued
    # staggered, so uA(i+1)'s matmuls hide uA(i)'s Act-engine copy and the
    # conv matmuls never head-of-line block the PE on the ze handoff.
    def make_unit_a(s, name, ct):
        def unit_a():
            st = P[s % 2]
            zp = psA.tile([128, LT], FP, tag="mm512", name="zp")
            for kt8 in range(8):
                nc.tensor.matmul(zp[:], wt[(name, kt8)][:, bass.ts(ct, 128)],
                                 st[("x", kt8)][:],
                                 start=(kt8 == 0), stop=(kt8 == 7))
            ze = zpool.tile([128, KT - 1 + LT], BF, tag="zext", name="ze")
            nc.gpsimd.tensor_copy(ze[:, 0:KT - 1], carries[(name, ct)][:])
            nc.scalar.copy(ze[:, KT - 1:], zp[:])
            nc.gpsimd.tensor_copy(carries[(name, ct)][:],
                                  ze[:, LT:LT + KT - 1])
            st[("ze", name, ct)] = ze
        unit_a.has_pe = True
        return unit_a

    def make_unit_b(s, name, ct):
        def unit_b():
            st = P[s % 2]
            ze = st.pop(("ze", name, ct))
            zc = psA.tile([128, LT], FP, tag="mm512", name="zc")
            for j in range(KT):
                nc.tensor.matmul(zc[:], diag[name][:, ct, j, :],
                                 ze[:, j:j + LT],
                                 start=(j == 0), stop=(j == KT - 1))
            if name == "v":
                t = stripD.tile([128, LT], BF, tag=f"zs_v{ct}", name="zsv")
            else:
                t = stripT.tile([128, LT], BF, tag=f"zs_{name}{ct}", name="zs")
            nc.scalar.activation(t[:], zc[:], AF.Silu)
            st[(name, ct)] = t
            if name in ("q", "k"):
                sqt = stripT.tile([128, LT], BF, tag=f"sq_{name}{ct}",
                                  name="sqt")
                if name == "q":
                    nc.vector.tensor_tensor(sqt[:], t[:], t[:], op=OP.mult)
                else:
                    nc.gpsimd.tensor_mul(sqt[:], t[:], t[:])
                st[("sq", name, ct)] = sqt
        unit_b.has_pe = True
        return unit_b

    def make_norms(s):
        # rsqall column layout: col = h*12 + lt*3 + r with r in
        # {0: rsq_q, 1: rsq_k, 2: rsq_k*beta} so one strided broadcast DMA
        # per head can build the [ch, lt, r, tok] bc tile directly.
        def norms():
            st = P[s % 2]
            # shares the projection pool's banks: the l2 colsums are one
            # brief use per strip, freeing a 6th bank for the scan rotation
            psq = psA.tile([128, 512], FP, tag="mm512", name="psq")
            for h in range(2):
                for lt in range(CPS):
                    base = h * 12 + lt * 3
                    # accumulation groups must not interleave within a PSUM
                    # bank: a start=True re-marks the whole 2KB zero region,
                    # so an open group's accumulate step would read zeros
                    for ct2 in range(2):
                        nc.tensor.matmul(
                            psq[:, base:base + 1],
                            st[("sq", "q", 2 * h + ct2)][:, bass.ts(lt, 128)],
                            ones_col[:], start=(ct2 == 0), stop=(ct2 == 1))
                    # k colsum lands in both r=1 and r=2 so the quake pass
                    # runs on one contiguous [128, 24] block
                    for ct2 in range(2):
                        nc.tensor.matmul(
                            psq[:, base + 1:base + 3],
                            st[("sq", "k", 2 * h + ct2)][:, bass.ts(lt, 128)],
                            ones2[:], start=(ct2 == 0), stop=(ct2 == 1))
            rsqall = small.tile([128, 24], BF, tag="rsqall", name="rsqall")
            dve_rsqrt(24, rsqall[:], psq[:, 0:24], 1.0,
                      EPS_L2 * EPS_L2, "rsA", iters=2)
            st["rsqall"] = rsqall
        return norms

    def make_g(s, lt):
        def g_unit():
            st = P[s % 2]
            if lt == 0:
                st["gg"] = stripD.tile([128, CPS, 512], BF, tag="gg",
                                       name="gg")
            gp = psA.tile([128, 512], FP, tag="mm512", name="gp")
            for kt8 in range(8):
                nc.tensor.matmul(gp[:], st[("x", kt8)][:, bass.ts(lt, 128)],
                                 wt[("g", kt8)][:],
                                 start=(kt8 == 0), stop=(kt8 == 7))
            # the gate g*sigmoid(g) is exactly silu(g)
            nc.scalar.activation(st["gg"][:, lt, :], gp[:], AF.Silu)
        g_unit.has_pe = True
        return g_unit

    def make_tail0(s):
        def tail0():
            st = P[s % 2]
            rsqall = st["rsqall"]
            rsv = rsqall[:].rearrange("p (h l r) -> p h l r", h=2, r=3)
            nc.vector.tensor_tensor(
                rsv[:, :, :, 2], rsv[:, :, :, 1],
                st["betas"][:].rearrange("p l h -> p h l"), op=OP.mult)
            st["bc"] = {}
            # transpose on PE (bf16, 128 rows), then broadcast across the
            # 128 channel partitions with a DRAM bounce: contiguous copy
            # out, partition-stride-0 read back (SBUF DMA APs cannot cross
            # partitions and are capped at 3 dims).
            rpt = psS.tile([128, 512], BF, tag="scan", name="rpt")
            nc.tensor.matmul(rpt[0:24, 0:128], rsqall[:], ident_b[:],
                             is_transpose=True, start=True, stop=True)
            rr24 = small.tile([24, 128], BF, tag="rr24", name="rr24")
            nc.scalar.copy(rr24[:], rpt[0:24, 0:128])
            nc.scalar.dma_start(out=bcd[s % 2], in_=rr24[:])
            for h in range(2):
                bc = stripT.tile([128, CPS, 3, 128], BF, tag=f"bcast{h}",
                                 name="bc")
                nc.scalar.dma_start(
                    out=bc[:],
                    in_=bcd[s % 2, h * 12:(h + 1) * 12].rearrange(
                        "a b -> (a b)").partition_broadcast(128))
                st["bc"][h] = bc
            st["kqT"], st["khT"] = {}, {}
        return tail0

    def make_tail_h(s, h, ct2):
        def tail_h():
            st = P[s % 2]
            bc = st["bc"][h]
            ct = 2 * h + ct2

            def c4(ap):
                return ap.rearrange("p (a b) -> p a b", a=CPS)

            kq = stripD.tile([128, 2, LT], BF, tag=f"kqT{h}{ct2}",
                             name="kq")
            # strips 0 / NS-1 have no scan slack around their boundary:
            # route the kb product to DVE so the Pool queue isn't serial
            if s == 0 or s == NS - 1:
                nc.vector.tensor_tensor(c4(kq[:, 0, :]), c4(st[("k", ct)][:]),
                                        bc[:, :, 2, :], op=OP.mult)
            else:
                nc.gpsimd.tensor_mul(c4(kq[:, 0, :]), c4(st[("k", ct)][:]),
                                     bc[:, :, 2, :])
            nc.vector.tensor_tensor(c4(kq[:, 1, :]), c4(st[("q", ct)][:]),
                                    bc[:, :, 0, :], op=OP.mult)
            kh = stripD.tile([128, LT], BF, tag=f"khT{h}{ct2}", name="kh")
            nc.gpsimd.tensor_mul(c4(kh[:]), c4(st[("k", ct)][:]),
                                 bc[:, :, 1, :])
            st["kqT"][(h, ct2)] = kq
            st["khT"][(h, ct2)] = kh
        return tail_h

    def push_strip(s, xdma=True):
        if xdma:
            fill_q.append(make_xdma(s))
        else:
            P[s % 2].update(x0_tiles)
            P[s % 2]["betas"] = bt0
        # qkv units staggered in halves: uA(i+1) queued between uA(i) and
        # uB(i) so the PE always has projection matmuls while the Act-engine
        # ze copies land.
        units = [(n, ct) for n in ("q", "k") for ct in range(4)]
        prev = None
        for n, ct in units:
            fill_q.append(make_unit_a(s, n, ct))
            if prev is not None:
                fill_q.append(make_unit_b(s, *prev))
            prev = (n, ct)
        fill_q.append(make_unit_a(s, "v", 0))
        fill_q.append(make_unit_b(s, *prev))
        fill_q.append(make_norms(s))
        # interleave the (DVE/DMA-heavy) tail closures with the (PE-heavy)
        # v units so no engine sees a multi-us lump and the tail0 PE
        # transpose never queues directly behind a long DVE backlog; the
        # reserved last-two closures are g units, which nothing in the next
        # scan's first chunk depends on
        fill_q.append(make_unit_a(s, "v", 1))
        fill_q.append(make_unit_b(s, "v", 0))
        fill_q.append(make_tail0(s))
        vq = [("v", 2), ("v", 3)]
        seq = []
        for i, (h, ct2) in enumerate(((0, 0), (0, 1), (1, 0), (1, 1))):
            if i < 2:
                seq.append(make_unit_a(s, *vq[i]))
                seq.append(make_unit_b(s, "v", i + 1))
            elif i == 2:
                seq.append(make_unit_b(s, "v", 3))
                seq.append(make_g(s, 0))
            else:
                seq.append(make_g(s, 1))
            seq.append(make_tail_h(s, h, ct2))
        fill_q.extend(seq)
        for lt in range(2, CPS):
            fill_q.append(make_g(s, lt))

    # ---- scan ---------------------------------------------------------------
    # output flushes run TWO chunks behind the scan: the DVE rsqrt/gate
    # chain of a strip's last chunk then hides behind the next strip's
    # first-chunk scan instead of stalling the PE at the boundary.
    pending = deque()

    def flush_pending(force=False, limit=None):
        n = 0
        while len(pending) >= (1 if force else 2):
            flush_one()
            n += 1
            if limit is not None and n >= limit:
                return

    def flush_one():
        plt, pl0, pogh = pending.popleft()
        # ogh was computed eagerly a chunk ago, so the PE transposes here
        # never wait on the DVE rsqrt/gate chain.
        otT = small.tile([128, 4, 128], BF, tag="otT")
        for h in range(2):
            for q2 in range(2):
                q4 = 2 * h + q2
                tp = psS.tile([128, 512], BF, tag="scan", name="tpf")
                nc.tensor.matmul(tp[:, 0:128], pogh[h][:, bass.ts(q2, 128)],
                                 ident_b[:], is_transpose=True,
                                 start=True, stop=True)
                if q4 % 2 == 0:
                    nc.scalar.copy(otT[:, q4, :], tp[:, 0:128])
                else:
                    nc.vector.tensor_copy(otT[:, q4, :], tp[:, 0:128])
        for nh in range(2):
            pop = psA.tile([128, 512], FP, tag="mm512", name="pop")
            for q4 in range(4):
                nc.tensor.matmul(pop[:], otT[:, q4, :],
                                 wo_t[:, q4, bass.ts(nh, 512)],
                                 start=(q4 == 0), stop=(q4 == 3))
            ou = small.tile([128, 512], FP, tag="outsb", name="ou")
            nc.scalar.copy(ou[:], pop[:])
            nc.sync.dma_start(
                out[pl0 + plt * 128:pl0 + (plt + 1) * 128,
                    nh * 512:(nh + 1) * 512], ou[:])

    Tout = [None, None]

    def scan_T(s, lt):
        # S-independent path for chunk lt: transposes, A products, masks,
        # Neumann inverse.  Emitted one chunk AHEAD so its cross-engine
        # handoffs hide behind the previous chunk's S path and fillers.
        st = P[s % 2]
        betas = st["betas"]
        kqT, khT = st["kqT"], st["khT"]
        cs = bass.ts(lt, 128)
        ktk, vb, ATat, Alow = {}, {}, {}, {}
        R, Lk, Uk = {}, {}, {}
        p1, p2, pp, pw = {}, {}, {}, {}
        for h in range(2):
            ktk[h] = hot.tile([128, 2, 128], BF, tag="ktok", name=f"ktk{h}")
            vb[h] = hot.tile([128, 256], BF, tag="vb", name=f"vb{h}")
            bcol = betas[:, lt, h:h + 1]
            for ct2 in range(2):
                tpk = psS.tile([128, 512], BF, tag="scan", name="tpk")
                nc.tensor.matmul(tpk[:, 0:128], khT[(h, ct2)][:, cs],
                                 ident_b[:], is_transpose=True,
                                 start=True, stop=True)
                tpv = psS.tile([128, 512], BF, tag="scan", name="tpv")
                nc.tensor.matmul(tpv[:, 0:128],
                                 st[("v", 2 * h + ct2)][:, cs],
                                 ident_b[:], is_transpose=True,
                                 start=True, stop=True)
                nc.vector.tensor_copy(ktk[h][:, ct2, :], tpk[:, 0:128])
                nc.scalar.mul(vb[h][:, bass.ts(ct2, 128)], tpv[:, 0:128],
                              bcol)
        for h in range(2):
            p1[h] = psS.tile([128, 512], FP, tag="scan", name="p1")
            for ct2 in range(2):
                nc.tensor.matmul(p1[h][:, 0:256], khT[(h, ct2)][:, cs],
                                 kqT[(h, ct2)][:, :, cs],
                                 start=(ct2 == 0), stop=(ct2 == 1))
            p2[h] = psS.tile([128, 512], FP, tag="scan", name="p2")
            for ct2 in range(2):
                nc.tensor.matmul(p2[h][:, 0:128], kqT[(h, ct2)][:, 0, cs],
                                 khT[(h, ct2)][:, cs],
                                 start=(ct2 == 0), stop=(ct2 == 1))
        for h in range(2):
            ATat[h] = hot.tile([128, 256], BF, tag="ATat", name=f"ATat{h}")
            nc.vector.tensor_tensor(ATat[h][:], p1[h][:, 0:256],
                                    mask_ua.rearrange("p a b -> p (a b)"),
                                    op=OP.mult)
            Alow[h] = hot.tile([128, 128], BF, tag="Alow", name=f"Alow{h}")
            nc.vector.tensor_tensor(Alow[h][:], p2[h][:, 0:128],
                                    mask_sl[:], op=OP.mult)
            R[h] = hot.tile([128, 128], BF, tag="Rn", name=f"R{h}")
            nc.vector.tensor_tensor(R[h][:], ident_b[:], ATat[h][:, 0:128],
                                    op=OP.subtract)
            Lk[h] = Alow[h][:, 0:128]
            Uk[h] = ATat[h][:, 0:128]
        fill(3)
        # Neumann: (I - A_T)(I + A_T^2)(I + A_T^4), squarings and lagged
        # R-updates in one psum tile/copy per level per head
        for lev in range(NLEV):
            for h in range(2):
                pp[h] = psS.tile([128, 512], FP, tag="scan", name="pp")
                nc.tensor.matmul(pp[h][:, 0:128], Uk[h], Lk[h],
                                 start=True, stop=True)
                nc.tensor.matmul(pp[h][:, 128:256], Lk[h], Uk[h],
                                 start=True, stop=True)
                if lev > 0:
                    nc.tensor.matmul(pp[h][:, 256:384], Lk[h], R[h][:],
                                     start=True, stop=True)
            for h in range(2):
                LUR = hot.tile([128, 384], BF, tag="LUR", name=f"LUR{h}")
                if (lev + h) % 2 == 0:
                    nc.scalar.copy(LUR[:, 0:256], pp[h][:, 0:256])
                else:
                    nc.vector.tensor_copy(LUR[:, 0:256], pp[h][:, 0:256])
                if lev > 0:
                    # R <- A^{2^lev} R + R: add the old R while draining psum
                    nc.vector.scalar_tensor_tensor(
                        LUR[:, 256:384], pp[h][:, 256:384], 1.0, R[h][:],
                        op0=OP.mult, op1=OP.add)
                    R[h] = LUR[:, 256:384]
                Lk[h] = LUR[:, 0:128]
                Uk[h] = LUR[:, 128:256]
            fill(2)
        Rf = {}
        for h in range(2):
            pw[h] = psS.tile([128, 512], FP, tag="scan", name="pw")
            # final factor: R <- (I + A_T^{2^NLEV}) R
            nc.tensor.matmul(pw[h][:, 256:384], Lk[h], R[h][:],
                             start=True, stop=True)
        for h in range(2):
            Rf[h] = hot.tile([128, 128], BF, tag="Rf", name=f"Rf{h}")
            nc.vector.scalar_tensor_tensor(Rf[h][:], pw[h][:, 256:384], 1.0,
                                           R[h][:], op0=OP.mult, op1=OP.add)
        fill()
        Tout[lt % 2] = (ktk, vb, ATat, Rf)

    def scan_S(s, lt):
        # S-dependent path: kb@S residual, U = T(vb - kb S), outputs and
        # the state update.  Consumes Tout (whose T path was emitted a
        # chunk ahead, so its cross-engine chain is already drained).
        st = P[s % 2]
        l0 = s * LT
        kqT = st["kqT"]
        ktk, vb, ATat, Rf = Tout[lt % 2]
        cs = bass.ts(lt, 128)
        pkS, rhs2 = {}, {}
        for h in range(2):
            pkS[h] = psS.tile([128, 512], FP, tag="scan", name="pkS")
            for ct2 in range(2):
                nc.tensor.matmul(pkS[h][:, 0:256], kqT[(h, ct2)][:, 0, cs],
                                 Sbf[h][:, ct2, :],
                                 start=(ct2 == 0), stop=(ct2 == 1))
        for h in range(2):
            rhs2[h] = hot.tile([128, 256], BF, tag="rhs2", name=f"rhs2{h}")
            nc.vector.tensor_tensor(rhs2[h][:], vb[h][:], pkS[h][:, 0:256],
                                    op=OP.subtract)
        fill(1)
        ssq2 = small.tile([128, 2], FP, tag="ssq_o", name="ssq2")
        Ut, pos, pu = {}, {}, {}
        for h in range(2):
            # U = T (vb - kb S), single matmul off the bf16 residual
            pu[h] = psS.tile([128, 512], FP, tag="scan", name="pu")
            nc.tensor.matmul(pu[h][:, 0:256], Rf[h][:], rhs2[h][:],
                             start=True, stop=True)
        for h in range(2):
            Ut[h] = hot.tile([128, 256], BF, tag="Ut", name=f"Ut{h}")
            if h == 0:
                nc.vector.tensor_copy(Ut[h][:], pu[h][:, 0:256])
            else:
                nc.scalar.copy(Ut[h][:], pu[h][:, 0:256])
        for h in range(2):
            po = psS.tile([128, 512], FP, tag="scan", name="po")
            for half in range(2):
                nc.tensor.matmul(po[:, 0:256], kqT[(h, half)][:, 1, cs],
                                 Sbf[h][:, half, :],
                                 start=(half == 0), stop=False)
            nc.tensor.matmul(po[:, 0:256], ATat[h][:, 128:256], Ut[h][:],
                             start=False, stop=True)
            pos[h] = po
            psu = psS.tile([128, 512], FP, tag="scan", name="psu")
            for half in range(2):
                nc.tensor.matmul(psu[:, bass.ts(half, 256)],
                                 ktk[h][:, half, :], Ut[h][:],
                                 start=True, stop=True)
            g = s * CPS + lt
            s_in = S32[h][g % 2][:].rearrange("p a b -> p (a b)")
            s_out = S32[h][1 - g % 2][:].rearrange("p a b -> p (a b)")
            nc.vector.tensor_tensor(s_out, psu[:, 0:512], s_in, op=OP.add)
            for half in range(2):
                nc.gpsimd.tensor_copy(Sbf[h][:, half, :],
                                      S32[h][1 - g % 2][:, half, :])
            scrd = hot.tile([128, 256], BF, tag="scrd", name="scrd")
            nc.scalar.activation(scrd[:], po[:, 0:256], AF.Square,
                                 accum_out=ssq2[:, h:h + 1])
        fill()

        # gate compute (rsqrt chain + gate multiply) emitted EAGERLY so the
        # deferred flush's PE transposes find ogh ready; only the PE/DMA
        # tail (transpose + out-proj + store) is deferred a chunk.
        rv2 = small.tile([128, 2], FP, tag="rv_o", name="rv2")
        dve_rsqrt(2, rv2[:], ssq2[:], 1.0 / Dh, EPS_RMS, "rvq", iters=1)
        oghs = {}
        for h in range(2):
            ogh = hot.tile([128, 256], BF, tag="ogh", name=f"ogh{h}",
                           bufs=16)
            nc.vector.scalar_tensor_tensor(
                ogh[:], pos[h][:, 0:256], rv2[:, h:h + 1],
                st["gg"][:, lt, bass.ts(h, 256)], op0=OP.mult, op1=OP.mult)
            oghs[h] = ogh
        pending.append((lt, l0, oghs))

    # ---- main loop ----------------------------------------------------------
    # scan_T for chunk g+1 is emitted BEFORE scan_S for chunk g, so chunk
    # g's S-chain (pkS -> rhs2 -> pu -> ... -> Sbf) overlaps chunk g+1's
    # T-chain (transposes, A products, Neumann) instead of serializing.
    push_strip(0, xdma=False)
    fill(100)
    TOT = NS * CPS
    scan_T(0, 0)
    for s in range(NS):
        n_left = len(fill_q)  # strip s closures reserved past its proj phase
        if s + 1 < NS:
            push_strip(s + 1)
        for lt in range(CPS):
            g = s * CPS + lt
            if lt == 0:
                fill_budget[0] = n_left
                fill(n_left)
            # strip NS-2 banks its output flushes; strip NS-1 drains the
            # backlog two per chunk so the pipeline tail has PE work
            if s == NS - 2:
                pass
            elif s == NS - 1:
                flush_pending(force=True, limit=2)
            else:
                flush_pending()
            rem = len(fill_q) - (2 if s + 1 < NS else 0)
            # next strip's proj/tail closures must all be emitted before
            # chunk (s,3) emits scan_T(s+1,0): spread them over chunks 0-2
            den = max(1, CPS - 1 - lt) if s + 1 < NS else max(1, CPS - lt)
            fill_budget[0] = max(0, -(-rem // den))
            if g + 1 < TOT:
                if (g + 1) % CPS == 0:
                    # entering strip s+1: drain everything its chunk-0 T
                    # path needs (the 2 reserved g-units may stay queued)
                    fill_budget[0] = max(0, len(fill_q) - 2)
                    fill(fill_budget[0])
                scan_T((g + 1) // CPS, (g + 1) % CPS)
            scan_S(s, lt)
    flush_pending(force=True)


_CACHED_NC = None


def _build():
    global _CACHED_NC
    if _CACHED_NC is not None:
        return _CACHED_NC
    nc = bacc.Bacc("TRN2", target_bir_lowering=False, debug=False)
    io = {}
    io["x"] = nc.dram_tensor("x", [D, L], BF, kind="ExternalInput").ap()
    for nm, shp in (("wq", [D, DL]), ("wk", [D, DL]), ("wv", [D, DL]),
                    ("wg", [D, DL])):
        io[nm] = nc.dram_tensor(nm, shp, BF, kind="ExternalInput").ap()
    io["bt"] = nc.dram_tensor("bt", [128, NS, CPS, 2], FP,
                              kind="ExternalInput").ap()
    io["bcd"] = nc.dram_tensor("bcd", [2, 24, 128], BF, kind="Internal").ap()
    io["wo"] = nc.dram_tensor("wo", [DL, D], BF, kind="ExternalInput").ap()
    for nm in ("cq", "ck", "cv"):
        io[nm] = nc.dram_tensor(nm, [DL, KT], FP, kind="ExternalInput").ap()
    io["out"] = nc.dram_tensor("out", [L, D], FP, kind="ExternalOutput").ap()
    with tile.TileContext(nc) as tc, ExitStack() as ctx:
        deltanet_core(ctx, tc, io)
    nc.compile()
    _CACHED_NC = nc
    return nc


def kernel(hidden_states, Wq, Wk, Wv, Wb, Wg, Wo, conv_q, conv_k, conv_v,
           norm_w):
    import ml_dtypes
    bf = ml_dtypes.bfloat16
    x = np.ascontiguousarray(np.asarray(hidden_states, dtype=np.float32))
    Wo_s = np.asarray(Wo, np.float32) * np.tile(np.asarray(norm_w, np.float32),
                                                H)[:, None]
    Wb_f = np.asarray(Wb, np.float32)
    nc = _build()
    in_maps = []
    for c in range(8):
        b, hg = c // 2, c % 2
        cols = slice(hg * DL, (hg + 1) * DL)
        # beta = sigmoid(x @ Wb) is input-only; fold it on the host like
        # the norm_w scaling of Wo.  [L, 2] -> [128, NS, CPS, 2]
        logits = x[b] @ Wb_f[:, 2 * hg:2 * hg + 2]
        beta = 1.0 / (1.0 + np.exp(-logits))
        bt = beta.reshape(NS, CPS, 128, 2).transpose(2, 0, 1, 3)
        in_maps.append({
            "x": np.ascontiguousarray(x[b].T.astype(bf)),
            "wq": np.ascontiguousarray(np.asarray(Wq, np.float32)[:, cols].astype(bf)),
            "wk": np.ascontiguousarray(np.asarray(Wk, np.float32)[:, cols].astype(bf)),
            "wv": np.ascontiguousarray(np.asarray(Wv, np.float32)[:, cols].astype(bf)),
            "wg": np.ascontiguousarray(np.asarray(Wg, np.float32)[:, cols].astype(bf)),
            "bt": np.ascontiguousarray(bt),
            "wo": np.ascontiguousarray(Wo_s[cols, :].astype(bf)),
            "cq": np.ascontiguousarray(np.asarray(conv_q, np.float32)[cols]),
            "ck": np.ascontiguousarray(np.asarray(conv_k, np.float32)[cols]),
            "cv": np.ascontiguousarray(np.asarray(conv_v, np.float32)[cols]),
        })
    res = run_bass_kernel_spmd(nc, in_maps, core_ids=list(range(8)))
    outv = np.zeros((B, L, D), np.float32)
    for c in range(8):
        outv[c // 2] += res.results[c]["out"]
    return outv



# revision 49
# speedup vs baseline: 1.0386x; 1.0214x over previous
"""DeltaNet forward Trainium2 kernel (8-core SPMD, batch x head-pair sharded).

Sharding: core c -> batch b=c//2, head-group hg=c%2 (heads 2hg, 2hg+1 = columns
hg*512 : hg*512+512 of the q/k/v/o projections).  Each core computes a partial
[L, D] output (its two heads' contribution through the output projection); the
host sums the two partials per batch.  norm_w is folded into Wo on the host.

Per-core pipeline, software-pipelined at emission level: the projection work
for strip s+1 is queued as ~20 filler closures and drained at designated
stall points inside strip s's scan chunks, so the (in-order) PE stream has
gap-filler matmuls wherever the scan's cross-engine handoffs would stall it.

  x^T strip (DMA xbar-transposed load, bf16)
  -> q/k/v projections channel-major z^T = W^T x^T (bf16 matmuls)
  -> depthwise causal conv = 4 accumulating diag(w_tap) matmuls (bf16)
  -> silu as z * sigmoid(z): sigmoid on Act engine, multiply on Pool engine
     (keeps the Act engine on a single act-function table all kernel long)
  -> l2norm scales via ones-matmul column sums; rsqrt via DVE quake+Newton
  -> per 128-token chunk: delta-rule scan in bf16; (I+A)^-1 by Neumann product
     form; S in fp32 master + bf16 working copy
  -> RMS-norm (DVE rsqrt) + swish gate, output projection (fp32r)
"""

import sys

sys.path.insert(0, "/opt/trn_rl_repo")

from collections import deque
from contextlib import ExitStack

import numpy as np

import concourse.bass as bass
import concourse.tile as tile
from concourse import bacc, mybir
from concourse.bass_utils import run_bass_kernel_spmd

FP = mybir.dt.float32
FR = mybir.dt.float32r
BF = mybir.dt.bfloat16
I32 = mybir.dt.int32
AF = mybir.ActivationFunctionType
OP = mybir.AluOpType

B, L, D, H = 4, 4096, 1024, 4
Dh = 256          # head dim
DL = 512          # per-core channels (2 heads)
KT = 4            # conv taps
C = 128           # chunk length (our choice; the math is chunk-size invariant)
LT = 512          # L-strip size
NS = L // LT      # 8 strips
CPS = LT // C     # 4 chunks per strip
NLEV = 2          # Neumann levels: exact would be 6 (A^64); A is
                  # strongly contractive here (||A||~0.5), A^8 ~ 0
EPS_RMS = 1e-5
EPS_L2 = 1e-12
QK3 = 0x5F3759DF  # quake fast-rsqrt seed constant


def deltanet_core(ctx: ExitStack, tc: tile.TileContext, io: dict):
    nc = tc.nc
    x, wq, wk, wv, wg, bt, wo, cq, ck, cv, out = (
        io["x"], io["wq"], io["wk"], io["wv"], io["wg"], io["bt"], io["wo"],
        io["cq"], io["ck"], io["cv"], io["out"])
    bcd = io["bcd"]

    pool1 = ctx.enter_context(tc.tile_pool(name="consts", bufs=1))
    xpool = ctx.enter_context(tc.tile_pool(name="xp", bufs=2))
    zpool = ctx.enter_context(tc.tile_pool(name="zp", bufs=4))
    stripT = ctx.enter_context(tc.tile_pool(name="stripT", bufs=1))
    stripD = ctx.enter_context(tc.tile_pool(name="stripD", bufs=2))
    small = ctx.enter_context(tc.tile_pool(name="small", bufs=2))
    hot = ctx.enter_context(tc.tile_pool(name="hot", bufs=5))
    spool = ctx.enter_context(tc.tile_pool(name="state", bufs=1))
    psA = ctx.enter_context(tc.tile_pool(name="psA", bufs=2, space="PSUM"))
    psS = ctx.enter_context(tc.tile_pool(name="psS", bufs=6, space="PSUM"))

    # ---- resident constants -------------------------------------------------
    # wq first, then strip 0's x, so the first projection can start ~6us in
    # instead of waiting for the whole constant working set to land.
    wt = {}
    x0_tiles = {}
    for name, w in (("q", wq),):
        wr = w.rearrange("(t p) n -> p t n", p=128)
        for kt8 in range(8):
            t = pool1.tile([128, 512], BF, tag=f"w{name}{kt8}",
                           name=f"w{name}{kt8}")
            nc.sync.dma_start(t[:], wr[:, kt8, :])
            wt[(name, kt8)] = t
    xr0 = x.rearrange("(t p) l -> p t l", p=128)
    for kt8 in range(8):
        xk = xpool.tile([128, LT], BF, tag=f"xt{kt8}", name=f"xt{kt8}")
        # Act DMA ring: runs concurrently with the weight loads on SP's ring
        nc.scalar.dma_start(out=xk[:], in_=xr0[:, kt8, 0:LT])
        x0_tiles[("x", kt8)] = xk
    cw = {}
    for name, cz in (("q", cq), ("k", ck), ("v", cv)):
        t = pool1.tile([128, 4, KT], FP, tag=f"c{name}")
        nc.sync.dma_start(t[:], cz.rearrange("(t p) j -> p t j", p=128))
        cw[name] = t
    bt0 = pool1.tile([128, CPS, 2], FP, tag="bt0")
    nc.sync.dma_start(bt0[:], bt[:, 0])
    for name, w in (("k", wk), ("v", wv), ("g", wg)):
        wr = w.rearrange("(t p) n -> p t n", p=128)
        for kt8 in range(8):
            t = pool1.tile([128, 512], BF, tag=f"w{name}{kt8}",
                           name=f"w{name}{kt8}")
            nc.sync.dma_start(t[:], wr[:, kt8, :])
            wt[(name, kt8)] = t
    wo_t = pool1.tile([128, 4, 1024], BF, tag="wo")
    nc.sync.dma_start(wo_t[:], wo.rearrange("(t p) n -> p t n", p=128))

    # mask_ua[:, 0, :] strict-upper, mask_ua[:, 1, :] inclusive-upper:
    # keep where x - a - y < 0  (a=0 -> x<y, a=1 -> x<=y)
    mask_ua = pool1.tile([128, 2, 128], FP, tag="mask_ua")
    nc.gpsimd.memset(mask_ua, 1.0)
    nc.gpsimd.affine_select(out=mask_ua[:], in_=mask_ua[:],
                            compare_op=OP.is_gt, fill=0.0, base=0,
                            pattern=[[1, 2], [1, 128]], channel_multiplier=-1)
    mask_sl = pool1.tile([128, 128], FP, tag="mask_sl")
    nc.gpsimd.memset(mask_sl, 1.0)
    nc.gpsimd.affine_select(out=mask_sl[:], in_=mask_sl[:],
                            compare_op=OP.is_gt, fill=0.0, base=0,
                            pattern=[[-1, 128]], channel_multiplier=1)
    from concourse.masks import make_identity
    ident_b = pool1.tile([128, 128], BF, tag="identb")
    make_identity(nc, ident_b)

    ones_col = pool1.tile([128, 1], BF, tag="ones_col")
    nc.vector.memset(ones_col, 1.0)
    ones2 = pool1.tile([128, 2], BF, tag="ones2")
    nc.vector.memset(ones2, 1.0)

    # conv diag tiles (bf16): diag(w_tap) per (tensor, ch-tile, tap).
    diag = {}
    for name in ("q", "k", "v"):
        dt = pool1.tile([128, 4, KT, 128], BF, tag=f"diag{name}")
        diag[name] = dt
        for ct in range(4):
            for j in range(KT):
                nc.vector.tensor_scalar_mul(dt[:, ct, j, :], ident_b[:],
                                            cw[name][:, ct, j:j + 1])

    # ---- persistent state ---------------------------------------------------
    # S32 ping-pongs between two buffers per head: the update writes the
    # other buffer (reading the old one), so chunk g+1's update never WARs
    # against chunk g's Sbf drain and the S critical chain is add-to-add.
    S32 = [[spool.tile([128, 2, 256], FP, name=f"S32_{h}{b}",
                       tag=f"S32_{h}{b}") for b in range(2)]
           for h in range(2)]
    Sbf = [spool.tile([128, 2, 256], BF, name=f"Sbf_{h}", tag=f"Sbf_{h}")
           for h in range(2)]
    for h in range(2):
        nc.vector.memset(S32[h][0], 0.0)
        nc.vector.memset(Sbf[h], 0.0)
    carries = {}
    for name in ("q", "k", "v"):
        for ct in range(4):
            cr = spool.tile([128, KT - 1], BF, tag=f"carry_{name}{ct}")
            nc.vector.memset(cr, 0.0)
            carries[(name, ct)] = cr

    # ---- DVE fast rsqrt (quake seed + Newton) ------------------------------
    # out = 1/sqrt(in*scale + bias); `inp` may live in PSUM.  Keeps the Act
    # engine free of Abs_reciprocal_sqrt so one act table serves the kernel.
    def dve_rsqrt(n, out_ap, inp, scale, bias, tag, iters, eng=None):
        # eng=nc.gpsimd runs the whole chain on Pool (inp must be SBUF)
        eng = eng or nc.vector
        xb = small.tile([128, n], FP, tag=f"{tag}_x", name=f"{tag}_x")
        eng.tensor_scalar(xb[:], inp, scale, bias,
                          op0=OP.mult, op1=OP.add)
        yb = small.tile([128, n], FP, tag=f"{tag}_y", name=f"{tag}_y")
        eng.tensor_scalar(yb.bitcast(I32)[:], xb.bitcast(I32)[:], 1,
                          None, op0=OP.logical_shift_right)
        eng.tensor_scalar(yb.bitcast(I32)[:], yb.bitcast(I32)[:], -1,
                          QK3, op0=OP.mult, op1=OP.add)
        for it in range(iters):
            hb = small.tile([128, n], FP, tag=f"{tag}_h{it}",
                            name=f"{tag}_h{it}")
            eng.tensor_tensor(hb[:], yb[:], yb[:], op=OP.mult)
            eng.tensor_tensor(hb[:], hb[:], xb[:], op=OP.mult)
            eng.tensor_scalar(hb[:], hb[:], -0.5, 1.5,
                              op0=OP.mult, op1=OP.add)
            if it == iters - 1:
                eng.tensor_tensor(out_ap, yb[:], hb[:], op=OP.mult)
            else:
                y2 = small.tile([128, n], FP, tag=f"{tag}_y{it}",
                                name=f"{tag}_y{it}")
                eng.tensor_tensor(y2[:], yb[:], hb[:], op=OP.mult)
                yb = y2

    # ---- filler queue: strip s+1 proj work drained inside strip s scan -----
    # Metered per chunk so chunk 3 (and the strip boundary) still has filler
    # matmuls instead of the queue draining greedily in the first chunks.
    fill_q = deque()
    fill_budget = [10 ** 9]

    def fill(n=1):
        for _ in range(n):
            if not fill_q or fill_budget[0] <= 0:
                return
            fill_budget[0] -= 1
            fill_q.popleft()()

    def fill_pe(k=1):
        # emit closures until k PE-heavy ones landed: latency-critical call
        # sites need actual matmul work between two dependent PE ops, not a
        # DMA/elementwise closure
        while k > 0 and fill_q and fill_budget[0] > 0:
            fill_budget[0] -= 1
            cl = fill_q.popleft()
            cl()
            if getattr(cl, "has_pe", False):
                k -= 1

    # P[s % 2]: per-strip proj outputs (tiles), written by filler closures
    P = [dict(), dict()]

    def make_xdma(s):
        def xdma():
            st = P[s % 2]
            l0 = s * LT
            xr = x.rearrange("(t p) l -> p t l", p=128)
            for kt8 in range(8):
                xk = xpool.tile([128, LT], BF, tag=f"xt{kt8}", name=f"xt{kt8}")
                nc.sync.dma_start(out=xk[:], in_=xr[:, kt8, l0:l0 + LT])
                st[("x", kt8)] = xk
            betas = small.tile([128, CPS, 2], FP, tag="betas", name="betas")
            nc.sync.dma_start(out=betas[:], in_=bt[:, s])
            st["betas"] = betas
        return xdma

    # each projection unit is split in two closures (uA: projection matmuls
    # + conv-window copy; uB: conv matmuls + silu + squares) queued
    # staggered, so uA(i+1)'s matmuls hide uA(i)'s Act-engine copy and the
    # conv matmuls never head-of-line block the PE on the ze handoff.
    def make_unit_a(s, name, ct):
        def unit_a():
            st = P[s % 2]
            zp = psA.tile([128, LT], FP, tag="mm512", name="zp")
            for kt8 in range(8):
                nc.tensor.matmul(zp[:], wt[(name, kt8)][:, bass.ts(ct, 128)],
                                 st[("x", kt8)][:],
                                 start=(kt8 == 0), stop=(kt8 == 7))
            ze = zpool.tile([128, KT - 1 + LT], BF, tag="zext", name="ze")
            nc.gpsimd.tensor_copy(ze[:, 0:KT - 1], carries[(name, ct)][:])
            nc.scalar.copy(ze[:, KT - 1:], zp[:])
            nc.gpsimd.tensor_copy(carries[(name, ct)][:],
                                  ze[:, LT:LT + KT - 1])
            st[("ze", name, ct)] = ze
        unit_a.has_pe = True
        return unit_a

    def make_unit_b(s, name, ct):
        def unit_b():
            st = P[s % 2]
            ze = st.pop(("ze", name, ct))
            zc = psA.tile([128, LT], FP, tag="mm512", name="zc")
            for j in range(KT):
                nc.tensor.matmul(zc[:], diag[name][:, ct, j, :],
                                 ze[:, j:j + LT],
                                 start=(j == 0), stop=(j == KT - 1))
            if name == "v":
                t = stripD.tile([128, LT], BF, tag=f"zs_v{ct}", name="zsv")
            else:
                t = stripT.tile([128, LT], BF, tag=f"zs_{name}{ct}", name="zs")
            nc.scalar.activation(t[:], zc[:], AF.Silu)
            st[(name, ct)] = t
            if name in ("q", "k"):
                sqt = stripT.tile([128, LT], BF, tag=f"sq_{name}{ct}",
                                  name="sqt")
                if name == "q":
                    nc.vector.tensor_tensor(sqt[:], t[:], t[:], op=OP.mult)
                else:
                    nc.gpsimd.tensor_mul(sqt[:], t[:], t[:])
                st[("sq", name, ct)] = sqt
        unit_b.has_pe = True
        return unit_b

    def make_norms(s):
        # rsqall column layout: col = h*12 + lt*3 + r with r in
        # {0: rsq_q, 1: rsq_k, 2: rsq_k*beta} so one strided broadcast DMA
        # per head can build the [ch, lt, r, tok] bc tile directly.
        def norms():
            st = P[s % 2]
            # shares the projection pool's banks: the l2 colsums are one
            # brief use per strip, freeing a 6th bank for the scan rotation
            psq = psA.tile([128, 512], FP, tag="mm512", name="psq")
            for h in range(2):
                for lt in range(CPS):
                    base = h * 12 + lt * 3
                    # accumulation groups must not interleave within a PSUM
                    # bank: a start=True re-marks the whole 2KB zero region,
                    # so an open group's accumulate step would read zeros
                    for ct2 in range(2):
                        nc.tensor.matmul(
                            psq[:, base:base + 1],
                            st[("sq", "q", 2 * h + ct2)][:, bass.ts(lt, 128)],
                            ones_col[:], start=(ct2 == 0), stop=(ct2 == 1))
                    # k colsum lands in both r=1 and r=2 so the quake pass
                    # runs on one contiguous [128, 24] block
                    for ct2 in range(2):
                        nc.tensor.matmul(
                            psq[:, base + 1:base + 3],
                            st[("sq", "k", 2 * h + ct2)][:, bass.ts(lt, 128)],
                            ones2[:], start=(ct2 == 0), stop=(ct2 == 1))
            rsqall = small.tile([128, 24], BF, tag="rsqall", name="rsqall")
            dve_rsqrt(24, rsqall[:], psq[:, 0:24], 1.0,
                      EPS_L2 * EPS_L2, "rsA", iters=2)
            st["rsqall"] = rsqall
        return norms

    def make_g(s, lt):
        def g_unit():
            st = P[s % 2]
            if lt == 0:
                st["gg"] = stripD.tile([128, CPS, 512], BF, tag="gg",
                                       name="gg")
            gp = psA.tile([128, 512], FP, tag="mm512", name="gp")
            for kt8 in range(8):
                nc.tensor.matmul(gp[:], st[("x", kt8)][:, bass.ts(lt, 128)],
                                 wt[("g", kt8)][:],
                                 start=(kt8 == 0), stop=(kt8 == 7))
            # the gate g*sigmoid(g) is exactly silu(g)
            nc.scalar.activation(st["gg"][:, lt, :], gp[:], AF.Silu)
        g_unit.has_pe = True
        return g_unit

    def make_tail0(s):
        def tail0():
            st = P[s % 2]
            rsqall = st["rsqall"]
            rsv = rsqall[:].rearrange("p (h l r) -> p h l r", h=2, r=3)
            nc.vector.tensor_tensor(
                rsv[:, :, :, 2], rsv[:, :, :, 1],
                st["betas"][:].rearrange("p l h -> p h l"), op=OP.mult)
            st["bc"] = {}
            # transpose on PE (bf16, 128 rows), then broadcast across the
            # 128 channel partitions with a DRAM bounce: contiguous copy
            # out, partition-stride-0 read back (SBUF DMA APs cannot cross
            # partitions and are capped at 3 dims).
            rpt = psS.tile([128, 512], BF, tag="scan", name="rpt")
            nc.tensor.matmul(rpt[0:24, 0:128], rsqall[:], ident_b[:],
                             is_transpose=True, start=True, stop=True)
            rr24 = small.tile([24, 128], BF, tag="rr24", name="rr24")
            nc.scalar.copy(rr24[:], rpt[0:24, 0:128])
            nc.scalar.dma_start(out=bcd[s % 2], in_=rr24[:])
            for h in range(2):
                bc = stripT.tile([128, CPS, 3, 128], BF, tag=f"bcast{h}",
                                 name="bc")
                nc.scalar.dma_start(
                    out=bc[:],
                    in_=bcd[s % 2, h * 12:(h + 1) * 12].rearrange(
                        "a b -> (a b)").partition_broadcast(128))
                st["bc"][h] = bc
            st["kqT"], st["khT"] = {}, {}
        return tail0

    def make_tail_h(s, h, ct2):
        def tail_h():
            st = P[s % 2]
            bc = st["bc"][h]
            ct = 2 * h + ct2

            def c4(ap):
                return ap.rearrange("p (a b) -> p a b", a=CPS)

            kq = stripD.tile([128, 2, LT], BF, tag=f"kqT{h}{ct2}",
                             name="kq")
            # strips 0 / NS-1 have no scan slack around their boundary:
            # route the kb product to DVE so the Pool queue isn't serial
            if s == 0 or s == NS - 1:
                nc.vector.tensor_tensor(c4(kq[:, 0, :]), c4(st[("k", ct)][:]),
                                        bc[:, :, 2, :], op=OP.mult)
            else:
                nc.gpsimd.tensor_mul(c4(kq[:, 0, :]), c4(st[("k", ct)][:]),
                                     bc[:, :, 2, :])
            nc.vector.tensor_tensor(c4(kq[:, 1, :]), c4(st[("q", ct)][:]),
                                    bc[:, :, 0, :], op=OP.mult)
            kh = stripD.tile([128, LT], BF, tag=f"khT{h}{ct2}", name="kh")
            nc.gpsimd.tensor_mul(c4(kh[:]), c4(st[("k", ct)][:]),
                                 bc[:, :, 1, :])
            st["kqT"][(h, ct2)] = kq
            st["khT"][(h, ct2)] = kh
        return tail_h

    def push_strip(s, xdma=True):
        if xdma:
            fill_q.append(make_xdma(s))
        else:
            P[s % 2].update(x0_tiles)
            P[s % 2]["betas"] = bt0
        # qkv units staggered in halves: uA(i+1) queued between uA(i) and
        # uB(i) so the PE always has projection matmuls while the Act-engine
        # ze copies land.
        units = [(n, ct) for n in ("q", "k") for ct in range(4)]
        prev = None
        for n, ct in units:
            fill_q.append(make_unit_a(s, n, ct))
            if prev is not None:
                fill_q.append(make_unit_b(s, *prev))
            prev = (n, ct)
        fill_q.append(make_unit_a(s, "v", 0))
        fill_q.append(make_unit_b(s, *prev))
        fill_q.append(make_norms(s))
        # interleave the (DVE/DMA-heavy) tail closures with the (PE-heavy)
        # v units so no engine sees a multi-us lump and the tail0 PE
        # transpose never queues directly behind a long DVE backlog; the
        # reserved last-two closures are g units, which nothing in the next
        # scan's first chunk depends on
        fill_q.append(make_unit_a(s, "v", 1))
        fill_q.append(make_unit_b(s, "v", 0))
        fill_q.append(make_tail0(s))
        vq = [("v", 2), ("v", 3)]
        seq = []
        for i, (h, ct2) in enumerate(((0, 0), (0, 1), (1, 0), (1, 1))):
            if i < 2:
                seq.append(make_unit_a(s, *vq[i]))
                seq.append(make_unit_b(s, "v", i + 1))
            elif i == 2:
                seq.append(make_unit_b(s, "v", 3))
                seq.append(make_g(s, 0))
            else:
                seq.append(make_g(s, 1))
            seq.append(make_tail_h(s, h, ct2))
        fill_q.extend(seq)
        for lt in range(2, CPS):
            fill_q.append(make_g(s, lt))

    # ---- scan ---------------------------------------------------------------
    # output flushes run TWO chunks behind the scan: the DVE rsqrt/gate
    # chain of a strip's last chunk then hides behind the next strip's
    # first-chunk scan instead of stalling the PE at the boundary.
    pending = deque()

    def flush_pending(force=False, limit=None):
        n = 0
        while len(pending) >= (1 if force else 2):
            flush_one()
            n += 1
            if limit is not None and n >= limit:
                return

    def flush_one():
        plt, pl0, pogh = pending.popleft()
        # ogh was computed eagerly a chunk ago, so the PE transposes here
        # never wait on the DVE rsqrt/gate chain.
        otT = small.tile([128, 4, 128], BF, tag="otT")
        for h in range(2):
            for q2 in range(2):
                q4 = 2 * h + q2
                tp = psS.tile([128, 512], BF, tag="scan", name="tpf")
                nc.tensor.matmul(tp[:, 0:128], pogh[h][:, bass.ts(q2, 128)],
                                 ident_b[:], is_transpose=True,
                                 start=True, stop=True)
                if q4 % 2 == 0:
                    nc.scalar.copy(otT[:, q4, :], tp[:, 0:128])
                else:
                    nc.vector.tensor_copy(otT[:, q4, :], tp[:, 0:128])
        for nh in range(2):
            pop = psA.tile([128, 512], FP, tag="mm512", name="pop")
            for q4 in range(4):
                nc.tensor.matmul(pop[:], otT[:, q4, :],
                                 wo_t[:, q4, bass.ts(nh, 512)],
                                 start=(q4 == 0), stop=(q4 == 3))
            ou = small.tile([128, 512], FP, tag="outsb", name="ou")
            nc.scalar.copy(ou[:], pop[:])
            nc.sync.dma_start(
                out[pl0 + plt * 128:pl0 + (plt + 1) * 128,
                    nh * 512:(nh + 1) * 512], ou[:])

    Tout = [None, None]

    def gen_T(s, lt):
        # S-independent path for chunk lt: transposes, A products, masks,
        # Neumann inverse.  A generator: the driver interleaves its stages
        # with the previous chunk's S path so each cross-engine hop in this
        # ladder has the other ladder's matmuls in front of it on the PE.
        st = P[s % 2]
        betas = st["betas"]
        kqT, khT = st["kqT"], st["khT"]
        cs = bass.ts(lt, 128)
        ktk, vb, ATat, Alow = {}, {}, {}, {}
        R, Lk, Uk = {}, {}, {}
        p1, p2, pp, pw = {}, {}, {}, {}
        for h in range(2):
            ktk[h] = hot.tile([128, 2, 128], BF, tag="ktok", name=f"ktk{h}")
            vb[h] = hot.tile([128, 256], BF, tag="vb", name=f"vb{h}")
            bcol = betas[:, lt, h:h + 1]
            for ct2 in range(2):
                tpk = psS.tile([128, 512], BF, tag="scan", name="tpk")
                nc.tensor.matmul(tpk[:, 0:128], khT[(h, ct2)][:, cs],
                                 ident_b[:], is_transpose=True,
                                 start=True, stop=True)
                tpv = psS.tile([128, 512], BF, tag="scan", name="tpv")
                nc.tensor.matmul(tpv[:, 0:128],
                                 st[("v", 2 * h + ct2)][:, cs],
                                 ident_b[:], is_transpose=True,
                                 start=True, stop=True)
                nc.vector.tensor_copy(ktk[h][:, ct2, :], tpk[:, 0:128])
                nc.scalar.mul(vb[h][:, bass.ts(ct2, 128)], tpv[:, 0:128],
                              bcol)
        for h in range(2):
            p1[h] = psS.tile([128, 512], FP, tag="scan", name="p1")
            for ct2 in range(2):
                nc.tensor.matmul(p1[h][:, 0:256], khT[(h, ct2)][:, cs],
                                 kqT[(h, ct2)][:, :, cs],
                                 start=(ct2 == 0), stop=(ct2 == 1))
            p2[h] = psS.tile([128, 512], FP, tag="scan", name="p2")
            for ct2 in range(2):
                nc.tensor.matmul(p2[h][:, 0:128], kqT[(h, ct2)][:, 0, cs],
                                 khT[(h, ct2)][:, cs],
                                 start=(ct2 == 0), stop=(ct2 == 1))
        for h in range(2):
            ATat[h] = hot.tile([128, 256], BF, tag="ATat", name=f"ATat{h}")
            nc.vector.tensor_tensor(ATat[h][:], p1[h][:, 0:256],
                                    mask_ua.rearrange("p a b -> p (a b)"),
                                    op=OP.mult)
            Alow[h] = hot.tile([128, 128], BF, tag="Alow", name=f"Alow{h}")
            nc.vector.tensor_tensor(Alow[h][:], p2[h][:, 0:128],
                                    mask_sl[:], op=OP.mult)
            R[h] = hot.tile([128, 128], BF, tag="Rn", name=f"R{h}")
            nc.vector.tensor_tensor(R[h][:], ident_b[:], ATat[h][:, 0:128],
                                    op=OP.subtract)
            Lk[h] = Alow[h][:, 0:128]
            Uk[h] = ATat[h][:, 0:128]
        fill(3)
        yield
        # Neumann: (I - A_T)(I + A_T^2)(I + A_T^4), squarings and lagged
        # R-updates in one psum tile/copy per level per head
        for lev in range(NLEV):
            for h in range(2):
                pp[h] = psS.tile([128, 512], FP, tag="scan", name="pp")
                nc.tensor.matmul(pp[h][:, 0:128], Uk[h], Lk[h],
                                 start=True, stop=True)
                nc.tensor.matmul(pp[h][:, 128:256], Lk[h], Uk[h],
                                 start=True, stop=True)
                if lev > 0:
                    nc.tensor.matmul(pp[h][:, 256:384], Lk[h], R[h][:],
                                     start=True, stop=True)
            for h in range(2):
                LUR = hot.tile([128, 384], BF, tag="LUR", name=f"LUR{h}")
                if (lev + h) % 2 == 0:
                    nc.scalar.copy(LUR[:, 0:256], pp[h][:, 0:256])
                else:
                    nc.vector.tensor_copy(LUR[:, 0:256], pp[h][:, 0:256])
                if lev > 0:
                    # R <- A^{2^lev} R + R: add the old R while draining psum
                    nc.vector.scalar_tensor_tensor(
                        LUR[:, 256:384], pp[h][:, 256:384], 1.0, R[h][:],
                        op0=OP.mult, op1=OP.add)
                    R[h] = LUR[:, 256:384]
                Lk[h] = LUR[:, 0:128]
                Uk[h] = LUR[:, 128:256]
            fill(2)
            yield
        Rf = {}
        for h in range(2):
            pw[h] = psS.tile([128, 512], FP, tag="scan", name="pw")
            # final factor: R <- (I + A_T^{2^NLEV}) R
            nc.tensor.matmul(pw[h][:, 256:384], Lk[h], R[h][:],
                             start=True, stop=True)
        for h in range(2):
            Rf[h] = hot.tile([128, 128], BF, tag="Rf", name=f"Rf{h}")
            nc.vector.scalar_tensor_tensor(Rf[h][:], pw[h][:, 256:384], 1.0,
                                           R[h][:], op0=OP.mult, op1=OP.add)
        fill()
        Tout[lt % 2] = (ktk, vb, ATat, Rf)

    def gen_S(s, lt):
        # S-dependent path: kb@S residual, U = T(vb - kb S), outputs and
        # the state update.  Consumes Tout (whose T path was emitted a
        # chunk ahead); run interleaved with the next chunk's T path.
        st = P[s % 2]
        l0 = s * LT
        kqT = st["kqT"]
        ktk, vb, ATat, Rf = Tout[lt % 2]
        cs = bass.ts(lt, 128)
        pkS, rhs2 = {}, {}
        for h in range(2):
            pkS[h] = psS.tile([128, 512], FP, tag="scan", name="pkS")
            for ct2 in range(2):
                nc.tensor.matmul(pkS[h][:, 0:256], kqT[(h, ct2)][:, 0, cs],
                                 Sbf[h][:, ct2, :],
                                 start=(ct2 == 0), stop=(ct2 == 1))
        for h in range(2):
            rhs2[h] = hot.tile([128, 256], BF, tag="rhs2", name=f"rhs2{h}")
            nc.vector.tensor_tensor(rhs2[h][:], vb[h][:], pkS[h][:, 0:256],
                                    op=OP.subtract)
        fill(1)
        yield
        ssq2 = small.tile([128, 2], FP, tag="ssq_o", name="ssq2")
        Ut, pos, pu = {}, {}, {}
        for h in range(2):
            # U = T (vb - kb S), single matmul off the bf16 residual
            pu[h] = psS.tile([128, 512], FP, tag="scan", name="pu")
            nc.tensor.matmul(pu[h][:, 0:256], Rf[h][:], rhs2[h][:],
                             start=True, stop=True)
        for h in range(2):
            Ut[h] = hot.tile([128, 256], BF, tag="Ut", name=f"Ut{h}")
            if h == 0:
                nc.vector.tensor_copy(Ut[h][:], pu[h][:, 0:256])
            else:
                nc.scalar.copy(Ut[h][:], pu[h][:, 0:256])
        yield
        for h in range(2):
            po = psS.tile([128, 512], FP, tag="scan", name="po")
            for half in range(2):
                nc.tensor.matmul(po[:, 0:256], kqT[(h, half)][:, 1, cs],
                                 Sbf[h][:, half, :],
                                 start=(half == 0), stop=False)
            nc.tensor.matmul(po[:, 0:256], ATat[h][:, 128:256], Ut[h][:],
                             start=False, stop=True)
            pos[h] = po
            psu = psS.tile([128, 512], FP, tag="scan", name="psu")
            for half in range(2):
                nc.tensor.matmul(psu[:, bass.ts(half, 256)],
                                 ktk[h][:, half, :], Ut[h][:],
                                 start=True, stop=True)
            g = s * CPS + lt
            s_in = S32[h][g % 2][:].rearrange("p a b -> p (a b)")
            s_out = S32[h][1 - g % 2][:].rearrange("p a b -> p (a b)")
            nc.vector.tensor_tensor(s_out, psu[:, 0:512], s_in, op=OP.add)
            for half in range(2):
                nc.gpsimd.tensor_copy(Sbf[h][:, half, :],
                                      S32[h][1 - g % 2][:, half, :])
            scrd = hot.tile([128, 256], BF, tag="scrd", name="scrd")
            nc.scalar.activation(scrd[:], po[:, 0:256], AF.Square,
                                 accum_out=ssq2[:, h:h + 1])
            if h == 0:
                yield
        fill()
        yield

        # gate compute (rsqrt chain + gate multiply) emitted EAGERLY so the
        # deferred flush's PE transposes find ogh ready; only the PE/DMA
        # tail (transpose + out-proj + store) is deferred a chunk.
        rv2 = small.tile([128, 2], FP, tag="rv_o", name="rv2")
        dve_rsqrt(2, rv2[:], ssq2[:], 1.0 / Dh, EPS_RMS, "rvq", iters=1)
        oghs = {}
        for h in range(2):
            ogh = hot.tile([128, 256], BF, tag="ogh", name=f"ogh{h}",
                           bufs=16)
            nc.vector.scalar_tensor_tensor(
                ogh[:], pos[h][:, 0:256], rv2[:, h:h + 1],
                st["gg"][:, lt, bass.ts(h, 256)], op0=OP.mult, op1=OP.mult)
            oghs[h] = ogh
        pending.append((lt, l0, oghs))

    # ---- main loop ----------------------------------------------------------
    # scan_T for chunk g+1 is emitted BEFORE scan_S for chunk g, so chunk
    # g's S-chain (pkS -> rhs2 -> pu -> ... -> Sbf) overlaps chunk g+1's
    # T-chain (transposes, A products, Neumann) instead of serializing.
    push_strip(0, xdma=False)
    fill(100)
    TOT = NS * CPS
    for _ in gen_T(0, 0):
        pass
    for s in range(NS):
        n_left = len(fill_q)  # strip s closures reserved past its proj phase
        if s + 1 < NS:
            push_strip(s + 1)
        for lt in range(CPS):
            g = s * CPS + lt
            if lt == 0:
                fill_budget[0] = n_left
                fill(n_left)
            # strip NS-2 banks its output flushes; strip NS-1 drains the
            # backlog two per chunk so the pipeline tail has PE work
            if s == NS - 2:
                pass
            elif s == NS - 1:
                flush_pending(force=True, limit=2)
            else:
                flush_pending()
            rem = len(fill_q) - (2 if s + 1 < NS else 0)
            # next strip's proj/tail closures must all be emitted before
            # chunk (s,3) emits scan_T(s+1,0): spread them over chunks 0-2
            den = max(1, CPS - 1 - lt) if s + 1 < NS else max(1, CPS - lt)
            fill_budget[0] = max(0, -(-rem // den))
            tT = None
            if g + 1 < TOT:
                if (g + 1) % CPS == 0:
                    # entering strip s+1: drain everything its chunk-0 T
                    # path needs (the 2 reserved g-units may stay queued)
                    fill_budget[0] = max(0, len(fill_q) - 2)
                    fill(fill_budget[0])
                tT = gen_T((g + 1) // CPS, (g + 1) % CPS)
            tS = gen_S(s, lt)
            for gen in (tT, tS, tT, tS, tT, tS, tT, tS, tS):
                if gen is not None:
                    next(gen, None)
    flush_pending(force=True)


_CACHED_NC = None


def _build():
    global _CACHED_NC
    if _CACHED_NC is not None:
        return _CACHED_NC
    nc = bacc.Bacc("TRN2", target_bir_lowering=False, debug=False)
    io = {}
    io["x"] = nc.dram_tensor("x", [D, L], BF, kind="ExternalInput").ap()
    for nm, shp in (("wq", [D, DL]), ("wk", [D, DL]), ("wv", [D, DL]),
                    ("wg", [D, DL])):
        io[nm] = nc.dram_tensor(nm, shp, BF, kind="ExternalInput").ap()
    io["bt"] = nc.dram_tensor("bt", [128, NS, CPS, 2], FP,
                              kind="ExternalInput").ap()
    io["bcd"] = nc.dram_tensor("bcd", [2, 24, 128], BF, kind="Internal").ap()
    io["wo"] = nc.dram_tensor("wo", [DL, D], BF, kind="ExternalInput").ap()
    for nm in ("cq", "ck", "cv"):
        io[nm] = nc.dram_tensor(nm, [DL, KT], FP, kind="ExternalInput").ap()
    io["out"] = nc.dram_tensor("out", [L, D], FP, kind="ExternalOutput").ap()
    with tile.TileContext(nc) as tc, ExitStack() as ctx:
        deltanet_core(ctx, tc, io)
    nc.compile()
    _CACHED_NC = nc
    return nc


def kernel(hidden_states, Wq, Wk, Wv, Wb, Wg, Wo, conv_q, conv_k, conv_v,
           norm_w):
    import ml_dtypes
    bf = ml_dtypes.bfloat16
    x = np.ascontiguousarray(np.asarray(hidden_states, dtype=np.float32))
    Wo_s = np.asarray(Wo, np.float32) * np.tile(np.asarray(norm_w, np.float32),
                                                H)[:, None]
    Wb_f = np.asarray(Wb, np.float32)
    nc = _build()
    in_maps = []
    for c in range(8):
        b, hg = c // 2, c % 2
        cols = slice(hg * DL, (hg + 1) * DL)
        # beta = sigmoid(x @ Wb) is input-only; fold it on the host like
        # the norm_w scaling of Wo.  [L, 2] -> [128, NS, CPS, 2]
        logits = x[b] @ Wb_f[:, 2 * hg:2 * hg + 2]
        beta = 1.0 / (1.0 + np.exp(-logits))
        bt = beta.reshape(NS, CPS, 128, 2).transpose(2, 0, 1, 3)
        in_maps.append({
            "x": np.ascontiguousarray(x[b].T.astype(bf)),
            "wq": np.ascontiguousarray(np.asarray(Wq, np.float32)[:, cols].astype(bf)),
            "wk": np.ascontiguousarray(np.asarray(Wk, np.float32)[:, cols].astype(bf)),
            "wv": np.ascontiguousarray(np.asarray(Wv, np.float32)[:, cols].astype(bf)),
            "wg": np.ascontiguousarray(np.asarray(Wg, np.float32)[:, cols].astype(bf)),
            "bt": np.ascontiguousarray(bt),
            "wo": np.ascontiguousarray(Wo_s[cols, :].astype(bf)),
            "cq": np.ascontiguousarray(np.asarray(conv_q, np.float32)[cols]),
            "ck": np.ascontiguousarray(np.asarray(conv_k, np.float32)[cols]),
            "cv": np.ascontiguousarray(np.asarray(conv_v, np.float32)[cols]),
        })
    res = run_bass_kernel_spmd(nc, in_maps, core_ids=list(range(8)))
    outv = np.zeros((B, L, D), np.float32)
    for c in range(8):
        outv[c // 2] += res.results[c]["out"]
    return outv

